# revision 50
# baseline (speedup 1.0000x reference)
"""Trainium2 Bass kernel for a dense transformer block (B=2, T=2048, D=1024, H=16).

Sharding (8 NeuronCores, one chip):
  - Token-split everywhere except attention: core i owns 512 tokens (rows
    512i:512i+512 of the flattened [4096, 1024] activation).
  - Head-split attention: core i owns heads {2i, 2i+1}.
  - Collectives (all AllToAll; no AllGather since AG is charged on its 8x
    output):
      #1  Q,K   (each core computes QKV for its own tokens, all heads, then
                 redistributes per-head)    [8, 256, 512] bf16
      #2  V     (token-major per k-tile)    [8, 512, 128] bf16
      #3  attention outputs back to token owners  [8, 128, 512] bf16
    #2 overlaps score compute; #1 overlaps V compute + residual transposes.

Layout is feature-major ("transposed", [feature, token]) on-chip; transposes
run on the PE (idle during the front-end) with DVE/ACT copies out of PSUM.
LN1 is folded into wqkv (w) and per-partition biases (b); LN2 is applied to
the residual stream BEFORE fc1 (w folded into wfc1, b as gelu bias), so the
MLP needs no per-h-tile unfold work on DVE. DMAs are few and large, spread
across the SP/ACT/Pool queues by dependency class (each queue sustains only
~1 transfer per ~3.6us in the cost model, and a DMA waiting on a collective
blocks everything behind it on the same queue).
"""

import numpy as np
import ml_dtypes

import concourse.bass as bass
import concourse.mybir as mybir
import concourse.tile as tile
from concourse import bacc
from concourse import bass_utils

F32 = mybir.dt.float32
MM_DT = mybir.dt.bfloat16
MM_NP = ml_dtypes.bfloat16

P = 128            # partitions
TB = 512           # tokens per core
D = 1024           # model dim
CT = D // P        # 8 feature tiles
NC = 8             # cores
TOKS = 2 * 2048    # global tokens
FF = 4096          # mlp hidden
DH = 64            # head dim
VW = 2 * (DH + 1)  # per-k-tile V columns: 2 heads x (64 dims + 1 ones col)
NKT = 32           # global k-tiles of 128 tokens

AF = mybir.ActivationFunctionType
OP = mybir.AluOpType
RG = [list(range(NC))]

ET_BUFS = 52       # et lookahead window (scores run ahead of AV by this much)

_cache: dict = {}


def _build():
    nc = bacc.Bacc(
        "TRN2",
        target_bir_lowering=False,
        debug=False,
        enable_asserts=False,
        num_devices=NC,
    )

    # ---- kernel I/O ----
    x_own = nc.dram_tensor("x_own", [TB, D], MM_DT, kind="ExternalInput").ap()
    wqkv = nc.dram_tensor("wqkv", [D, 3 * D], MM_DT, kind="ExternalInput").ap()
    wproj = nc.dram_tensor("wproj", [D, D], MM_DT, kind="ExternalInput").ap()
    wfc1 = nc.dram_tensor("wfc1", [D, FF], MM_DT, kind="ExternalInput").ap()
    wfc2 = nc.dram_tensor("wfc2", [FF, D], MM_DT, kind="ExternalInput").ap()
    fc1B = nc.dram_tensor("fc1B", [P, FF // P], F32, kind="ExternalInput").ap()
    bqkv = nc.dram_tensor("bqkv", [1, 3 * D], MM_DT, kind="ExternalInput").ap()
    bqkvc = nc.dram_tensor("bqkvc", [P, 2 * CT], F32, kind="ExternalInput").ap()
    cmask2 = nc.dram_tensor("cmask2", [P, 2 * P], F32, kind="ExternalInput").ap()
    id128m = nc.dram_tensor("id128m", [P, P], MM_DT, kind="ExternalInput").ap()
    out_t = nc.dram_tensor("out_t", [D, TB], F32, kind="ExternalOutput").ap()

    with tile.TileContext(nc) as tc:
        with (
            tc.tile_pool(name="persist", bufs=1) as pers,
            tc.tile_pool(name="dram", bufs=1, space="DRAM") as dram,
        ):
            # collective buffers
            qk_in = dram.tile([NC, 2 * P, TB], MM_DT, name="qk_in", tag="qk_in")
            qk_out = dram.tile([NC, 2 * P, TB], MM_DT, name="qk_out", tag="qk_out")
            v_in = dram.tile([NC, TB, P], MM_DT, name="v_in", tag="v_in")
            v_out = dram.tile([NC, TB, P], MM_DT, name="v_out", tag="v_out")
            y_in = dram.tile([NC, P, TB], MM_DT, name="y_in", tag="y_in")
            y_out = dram.tile([NC, P, TB], MM_DT, name="y_out", tag="y_out")

            # small constants
            ones_col_m = pers.tile([P, 1], MM_DT, name="ones_col_m", tag="ones_col_m")
            nc.gpsimd.memset(ones_col_m[:], 1.0)
            ones_row_m = pers.tile([1, DH], MM_DT, name="ones_row_m", tag="ones_row_m")
            nc.gpsimd.memset(ones_row_m[:], 1.0)
            ones_row_f = pers.tile([1, P], F32, name="ones_row_f", tag="ones_row_f")
            nc.gpsimd.memset(ones_row_f[:], 1.0)
            zb = pers.tile([P, 1], F32, name="zb", tag="zb")
            nc.gpsimd.memset(zb[:], 0.0)
            eps1 = pers.tile([1, 1], F32, name="eps1", tag="eps1")
            nc.gpsimd.memset(eps1[:], 1e-5)
            epsP = pers.tile([P, 1], F32, name="epsP", tag="epsP")
            nc.gpsimd.memset(epsP[:], 1e-5)
            cmask_sb = pers.tile([P, 2 * P], F32, name="cmask_sb", tag="cmask_sb")
            nc.gpsimd.dma_start(cmask_sb[:], cmask2[:])
            idm_sb = pers.tile([P, P], MM_DT, name="idm_sb", tag="idm_sb")
            nc.gpsimd.dma_start(idm_sb[:], id128m[:])
            fc1B_sb = pers.tile([P, FF // P], F32, name="fc1B_sb", tag="fc1B_sb")
            bqkvc_sb = pers.tile([P, 2 * CT], F32, name="bqkvc_sb", tag="bqkvc_sb")

            # persistent activations
            xT = pers.tile([P, CT * TB], F32, name="xT", tag="xT")  # raw x, feat-major
            x2T = [pers.tile([P, TB], F32, name=f"x2T{c}", tag=f"x2T{c}")
                   for c in range(CT)]
            qT = pers.tile([P, TOKS], MM_DT, name="qT", tag="qT")
            kT = pers.tile([P, TOKS], MM_DT, name="kT", tag="kT")
            V_sb = pers.tile([P, NKT * VW], MM_DT, name="V_sb", tag="V_sb")
            # ones columns (data columns DMA'd in later)
            nc.gpsimd.memset(
                V_sb[:].rearrange("p (g w) -> p g w", w=DH + 1)[:, :, DH:DH + 1], 1.0)
            x2m = [pers.tile([P, TB], MM_DT, name=f"x2m{c}", tag=f"x2m{c}")
                   for c in range(CT)]
            wproj_sb = pers.tile([P, CT * D], MM_DT, name="wproj_sb", tag="wproj_sb")

            # ============ Phase A: load, LN1, QKV, A2A #1/#2 ============
            with (
                tc.tile_pool(name="ps_a", bufs=6, space="PSUM") as ps_a,
                tc.tile_pool(name="ps_t", bufs=2, space="PSUM") as ps_t,
                tc.tile_pool(name="work_a", bufs=2) as work_a,
                tc.tile_pool(name="xrow_a", bufs=1) as xrow_a,
                tc.tile_pool(name="wqkv_a", bufs=1) as wqkv_a,
                tc.tile_pool(name="xn_a", bufs=1) as xn_a,
                tc.tile_pool(name="qkl_a", bufs=1) as qkl_a,
                tc.tile_pool(name="vl_a", bufs=1) as vl_a,
            ):
                # single big DMAs: per-queue throughput is the scarce resource
                bqkv_sb = xrow_a.tile([1, 3 * D], MM_DT, name="bqkv_sb",
                                      tag="bqkv_sb")
                ones_row_p = xrow_a.tile([1, P], MM_DT, name="ones_row_p",
                                         tag="ones_row_p")
                nc.gpsimd.memset(ones_row_p[:], 1.0)
                xrow = xrow_a.tile([P, 4 * D], MM_DT, name="xrow", tag="xrow")
                nc.sync.dma_start(
                    xrow[:].rearrange("p (r c) -> p r c", r=4),
                    x_own[:].rearrange("(r p) c -> p r c", p=P))
                wqkv_sb = wqkv_a.tile([P, CT * 3 * D], MM_DT, name="wqkv_sb",
                                      tag="wqkv_sb")

                def wq(c):  # [128, 3*D] block of k-tile c
                    return wqkv_sb[:, c * 3 * D:(c + 1) * 3 * D]

                for half in range(2):
                    nc.sync.dma_start(
                        wqkv_sb[:, half * 4 * 3 * D:(half + 1) * 4 * 3 * D]
                        .rearrange("p (c j) -> p c j", c=4),
                        wqkv[half * 4 * P:(half + 1) * 4 * P, :]
                        .rearrange("(c p) j -> p c j", p=P))
                nc.scalar.dma_start(bqkv_sb[:], bqkv[:])
                nc.scalar.dma_start(bqkvc_sb[:], bqkvc[:])
                nc.gpsimd.dma_start(fc1B_sb[:], fc1B[:])

                # LN1 per row-block (natural layout, bf16; ln1_w folded into
                # wqkv, ln1_b via bias terms). DVE-only chain except the Sqrt.
                xnn_l = [xn_a.tile([P, D], MM_DT, name=f"xnn{r}", tag=f"xnn{r}")
                         for r in range(TB // P)]
                for r in range(TB // P):
                    xr = xrow[:, r * D:(r + 1) * D]
                    s1 = work_a.tile([P, 1], F32, name=f"s1_{r}", tag="lns1")
                    nc.vector.tensor_reduce(s1[:], xr,
                                            axis=mybir.AxisListType.X, op=OP.add)
                    sqf = work_a.tile([P, D], MM_DT, name=f"sq_{r}", tag="lnsq")
                    s2 = work_a.tile([P, 1], F32, name=f"s2_{r}", tag="lns2")
                    nc.vector.scalar_tensor_tensor(
                        out=sqf[:], in0=xr, scalar=1.0, in1=xr,
                        op0=OP.mult, op1=OP.mult, accum_out=s2[:])
                    mu = work_a.tile([P, 1], F32, name=f"mu_{r}", tag="lnmu")
                    nc.vector.tensor_scalar_mul(mu[:], s1[:], 1.0 / D)
                    mu2 = work_a.tile([P, 1], F32, name=f"mu2_{r}", tag="lnmu2")
                    nc.vector.tensor_mul(mu2[:], mu[:], mu[:])
                    var = work_a.tile([P, 1], F32, name=f"var_{r}", tag="lnvar")
                    nc.vector.scalar_tensor_tensor(
                        out=var[:], in0=s2[:], scalar=1.0 / D, in1=mu2[:],
                        op0=OP.mult, op1=OP.subtract)
                    sd = work_a.tile([P, 1], F32, name=f"sd_{r}", tag="lnsd")
                    nc.scalar.activation(sd[:], var[:], AF.Sqrt, bias=epsP[:])
                    rstd = work_a.tile([P, 1], F32, name=f"rstd_{r}", tag="lnrstd")
                    nc.vector.reciprocal(rstd[:], sd[:])
                    nc.vector.tensor_scalar(xnn_l[r][:], xr, mu[:], rstd[:],
                                            OP.subtract, OP.mult)

                # transposes on the (idle) PE; copies split across DVE/ACT
                xnT = xn_a.tile([P, CT * TB], MM_DT, name="xnT", tag="xnT")
                for r in range(TB // P):
                    for c in range(CT):
                        pt = ps_t.tile([P, P], MM_DT, name=f"ptn_{r}_{c}",
                                       tag="ptn")
                        nc.tensor.transpose(
                            pt[:], xnn_l[r][:, c * P:(c + 1) * P], idm_sb[:])
                        nc.vector.tensor_copy(
                            xnT[:, c * TB + r * P:c * TB + (r + 1) * P], pt[:])

                # ---- Q,K matmuls (feature-major, my 512 tokens) ----
                # c-major in groups so the PE streams while later weight
                # chunks are still in flight
                qkl = qkl_a.tile([P, NC * 2 * TB], MM_DT, name="qkl", tag="qkl")
                for g in range(2):  # 0=q, 1=k
                    ps_l = [ps_a.tile([P, TB], F32, name=f"qk{d}_{g}", tag="psqk")
                            for d in range(NC)]
                    for c in range(CT):
                        for d in range(NC):
                            nc.tensor.matmul(
                                ps_l[d][:], wq(c)[:, g * D + d * P:
                                                  g * D + (d + 1) * P],
                                xnT[:, c * TB:(c + 1) * TB],
                                start=(c == 0), stop=(c == CT - 1))
                    for d in range(NC):
                        # ln1_b contribution (b @ W) as per-partition bias
                        nc.scalar.activation(
                            qkl[:, (2 * d + g) * TB:(2 * d + g + 1) * TB],
                            ps_l[d][:], AF.Identity,
                            bias=bqkvc_sb[:, g * CT + d:g * CT + d + 1])
                    if g == 0:
                        nc.scalar.dma_start(
                            qk_in[:, 0:P, :].rearrange("d p t -> p d t"),
                            qkl[:].rearrange("p (d gg t) -> p d gg t", gg=2, t=TB)
                            [:, :, 0, :])
                    else:
                        nc.sync.dma_start(
                            qk_in[:, P:2 * P, :].rearrange("d p t -> p d t"),
                            qkl[:].rearrange("p (d gg t) -> p d gg t", gg=2, t=TB)
                            [:, :, 1, :])
                nc.gpsimd.collective_compute(
                    "AllToAll", OP.bypass, replica_groups=RG,
                    ins=[qk_in[:]], outs=[qk_out[:]])

                # ---- V matmuls (token-major) + A2A #2 ----
                vloc = vl_a.tile([P, 4 * D], MM_DT, name="vloc", tag="vloc")
                for tt in range(TB // P):
                    for nh in range(2):
                        ps = ps_a.tile([P, TB], F32, name=f"v{tt}_{nh}", tag="psqk")
                        nc.tensor.matmul(
                            ps[:], ones_row_p[:],
                            bqkv_sb[0:1, 2 * D + nh * TB:2 * D + (nh + 1) * TB],
                            start=True, stop=False)
                        for c in range(CT):
                            nc.tensor.matmul(
                                ps[:],
                                xnT[:, c * TB + tt * P:c * TB + (tt + 1) * P],
                                wq(c)[:, 2 * D + nh * TB:2 * D + (nh + 1) * TB],
                                start=False, stop=(c == CT - 1))
                        # write dest-major: vloc[p, (d*4 + tt)*P + c]
                        nc.scalar.activation(
                            vloc[:].rearrange("p (q tt c) -> p q tt c",
                                              tt=4, c=P)
                            [:, nh * 4:(nh + 1) * 4, tt, :],
                            ps[:], AF.Copy)
                nc.sync.dma_start(
                    v_in[:].rearrange("d (tt p) c -> p (d tt) c", p=P),
                    vloc[:].rearrange("p (dt c) -> p dt c", c=P))
                nc.gpsimd.collective_compute(
                    "AllToAll", OP.bypass, replica_groups=RG,
                    ins=[v_in[:]], outs=[v_out[:]])

                # ---- residual-path transposes of raw x (fills A2A wait) ----
                for r in range(TB // P):
                    for c in range(CT):
                        pt = ps_t.tile([P, P], MM_DT, name=f"ptr_{r}_{c}",
                                       tag="ptn")
                        nc.tensor.transpose(
                            pt[:], xrow[:, r * D + c * P:r * D + (c + 1) * P],
                            idm_sb[:])
                        nc.scalar.activation(
                            xT[:, c * TB + r * P:c * TB + (r + 1) * P],
                            pt[:], AF.Copy)
                for half in range(2):
                    nc.scalar.dma_start(
                        wproj_sb[:, half * 4 * D:(half + 1) * 4 * D]
                        .rearrange("p (c j) -> p c j", c=4),
                        wproj[half * 4 * P:(half + 1) * 4 * P, :]
                        .rearrange("(c p) j -> p c j", p=P))

            # ============ Phase B: attention ============
            with (
                tc.tile_pool(name="ps_s", bufs=2, space="PSUM") as ps_s_pool,
                tc.tile_pool(name="ps_y", bufs=2, space="PSUM") as ps_y_pool,
                tc.tile_pool(name="epool", bufs=ET_BUFS) as epool,
                tc.tile_pool(name="work_b", bufs=4) as work_b,
                tc.tile_pool(name="rb_b", bufs=2) as rb_b,
            ):
                # receive q/k (feature-major, all tokens for my 2 heads);
                # batch-0 sources first so b=0 scores start immediately
                for half in range(2):
                    sl = slice(half * 4, (half + 1) * 4)
                    nc.sync.dma_start(
                        kT[:, half * 4 * TB:(half + 1) * 4 * TB]
                        .rearrange("p (s t) -> p s t", s=4),
                        qk_out[sl, P:2 * P, :].rearrange("s p t -> p s t"))
                    nc.scalar.dma_start(
                        qT[:, half * 4 * TB:(half + 1) * 4 * TB]
                        .rearrange("p (s t) -> p s t", s=4),
                        qk_out[sl, 0:P, :].rearrange("s p t -> p s t"))
                # receive V into packed [tok, (head, dh|one)] slots
                for h in range(2):
                    nc.sync.dma_start(
                        V_sb[:].rearrange("p (gk hh w) -> p gk hh w",
                                          w=DH + 1, hh=2)[:, :, h, 0:DH],
                        v_out[:].rearrange("s (tt p) (hh c) -> p (s tt) hh c",
                                           p=P, c=DH)[:, :, h, :])

                # score/exp jobs and AV jobs, interleaved with a lookahead
                # window so the PE never sits behind a V-gated AV while
                # score work remains.
                sjobs = []   # (b, j, kt, pss, et)
                for b in range(2):
                    for j in range(4):
                        for kt in range(4 * j + 4):
                            sjobs.append([b, j, kt, None, None])

                def emit_score(job):
                    b, j, kt, _, _ = job
                    jg = b * 4 + j
                    qoff = jg * TB
                    gk = b * 16 + kt
                    n0 = 0 if kt < 4 * j else (kt - 4 * j) * P
                    pss = ps_s_pool.tile([P, 2 * TB], F32,
                                         name=f"pss{b}_{j}_{kt}", tag="pss")
                    for h in range(2):
                        nc.tensor.matmul(
                            pss[:, h * TB + n0:(h + 1) * TB],
                            kT[h * DH:(h + 1) * DH, gk * P:(gk + 1) * P],
                            qT[h * DH:(h + 1) * DH, qoff + n0:qoff + TB],
                            start=True, stop=True,
                            tile_position=(h * DH, 0))
                    if kt >= 4 * j:
                        nc.vector.tensor_add(
                            pss[:].rearrange("p (h t) -> p h t", h=2)
                            [:, :, n0:n0 + P],
                            pss[:].rearrange("p (h t) -> p h t", h=2)
                            [:, :, n0:n0 + P],
                            cmask_sb[:].rearrange("p (h t) -> p h t", h=2))
                    et = epool.tile([P, 2 * TB], MM_DT,
                                    name=f"et{b}_{j}_{kt}", tag="et")
                    if n0 == 0:
                        nc.scalar.activation(et[:], pss[:], AF.Exp, bias=zb[:])
                    else:
                        nc.scalar.activation(
                            et[:].rearrange("p (h t) -> p h t", h=2)[:, :, n0:TB],
                            pss[:].rearrange("p (h t) -> p h t", h=2)[:, :, n0:TB],
                            AF.Exp, bias=zb[:])
                    job[3] = pss
                    job[4] = et

                si = 0
                for si in range(min(ET_BUFS - 2, len(sjobs))):
                    emit_score(sjobs[si])
                si += 1

                for b in range(2):
                    for j in range(4):
                        jg = b * 4 + j
                        nkt = 4 * j + 4
                        ps_y = [ps_y_pool.tile([DH + 1, TB], F32,
                                               name=f"psy{b}_{j}_{h}", tag=f"psy{h}")
                                for h in range(2)]
                        base = sum(4 * (g % 4) + 4 for g in range(4 * b + j))
                        for kt in range(nkt):
                            job = sjobs[base + kt]
                            _, _, _, pss, et = job
                            n0 = 0 if kt < 4 * j else (kt - 4 * j) * P
                            gk = b * 16 + kt
                            for h in range(2):
                                vsl = V_sb[:, gk * VW + h * (DH + 1):
                                           gk * VW + h * (DH + 1) + DH + 1]
                                nc.tensor.matmul(
                                    ps_y[h][:, n0:TB], vsl,
                                    et[:, h * TB + n0:(h + 1) * TB],
                                    start=(kt == 0), stop=(kt == nkt - 1))
                            if si < len(sjobs):
                                emit_score(sjobs[si])
                                si += 1
                        # normalize: y / colsum -> bf16 -> a2a staging
                        yn = work_b.tile([P, TB], MM_DT, name=f"yn{b}_{j}",
                                         tag="yn")
                        for h in range(2):
                            rf = work_b.tile([1, TB], F32, name=f"rf{b}_{j}_{h}",
                                             tag="rf")
                            nc.vector.reciprocal(rf[:], ps_y[h][DH:DH + 1, :])
                            # denominator broadcast on the idle GPSIMD
                            rb = rb_b.tile([DH, TB], F32, name=f"rb{b}_{j}_{h}",
                                           tag="rb")
                            nc.gpsimd.partition_broadcast(rb[:], rf[:],
                                                          channels=DH)
                            nc.vector.tensor_mul(
                                yn[h * DH:(h + 1) * DH, :],
                                ps_y[h][0:DH, :], rb[:])
                        nc.sync.dma_start(y_in[jg], yn[:])

                nc.gpsimd.collective_compute(
                    "AllToAll", OP.bypass, replica_groups=RG,
                    ins=[y_in[:]], outs=[y_out[:]])

            # ============ Phase C: proj + residual + LN2 ============
            with (
                tc.tile_pool(name="ps_c", bufs=2, space="PSUM") as ps_c,
                tc.tile_pool(name="ps_ln", bufs=2, space="PSUM") as ps_ln,
                tc.tile_pool(name="ps_bc", bufs=2, space="PSUM") as ps_bc,
                tc.tile_pool(name="work_c", bufs=2) as work_c,
                tc.tile_pool(name="yall_c", bufs=1) as yall_c,
            ):
                yall = yall_c.tile([P, NC * TB], MM_DT, name="yall", tag="yall")
                nc.scalar.dma_start(
                    yall[:, 0:4 * TB].rearrange("p (s t) -> p s t", s=4),
                    y_out[0:4].rearrange("s p t -> p s t"))
                nc.sync.dma_start(
                    yall[:, 4 * TB:].rearrange("p (s t) -> p s t", s=4),
                    y_out[4:8].rearrange("s p t -> p s t"))
                s1 = ps_ln.tile([1, TB], F32, name="ln2s1", tag="lnsum")
                s2 = ps_ln.tile([1, TB], F32, name="ln2s2", tag="lnsum")
                for c in range(CT):
                    ps = ps_c.tile([P, TB], F32, name=f"proj{c}", tag="projps")
                    for s in range(NC):
                        nc.tensor.matmul(
                            ps[:],
                            wproj_sb[:, s * D + c * P:s * D + (c + 1) * P],
                            yall[:, s * TB:(s + 1) * TB],
                            start=(s == 0), stop=(s == NC - 1))
                    nc.vector.tensor_add(x2T[c][:], ps[:],
                                         xT[:, c * TB:(c + 1) * TB])
                    # LN2 stats via ones-matmuls on a bf16 scratch copy
                    x2s = work_c.tile([P, TB], MM_DT, name=f"x2s{c}", tag="x2s")
                    nc.scalar.activation(x2s[:], x2T[c][:], AF.Copy)
                    nc.tensor.matmul(s1[:], ones_col_m[:], x2s[:],
                                     start=(c == 0), stop=(c == CT - 1))
                    sq = work_c.tile([P, TB], MM_DT, name=f"sq{c}", tag="ln2sq")
                    nc.vector.tensor_mul(sq[:], x2s[:], x2s[:])
                    nc.tensor.matmul(s2[:], ones_col_m[:], sq[:],
                                     start=(c == 0), stop=(c == CT - 1))
                mu = work_c.tile([1, TB], F32, name="ln2mu", tag="ln2mu")
                nc.vector.tensor_scalar_mul(mu[:], s1[:], 1.0 / D)
                mu2 = work_c.tile([1, TB], F32, name="ln2mu2", tag="ln2mu2")
                nc.vector.tensor_mul(mu2[:], mu[:], mu[:])
                vr = work_c.tile([1, TB], F32, name="ln2vr", tag="ln2vr")
                nc.vector.scalar_tensor_tensor(
                    out=vr[:], in0=s2[:], scalar=1.0 / D, in1=mu2[:],
                    op0=OP.mult, op1=OP.subtract)
                sdr = work_c.tile([1, TB], F32, name="ln2sd", tag="ln2sd")
                nc.scalar.activation(sdr[:], vr[:], AF.Sqrt, bias=eps1[:])
                rs = work_c.tile([1, TB], F32, name="ln2rs", tag="ln2rs")
                nc.vector.reciprocal(rs[:], sdr[:])
                mrow = work_c.tile([1, TB], F32, name="ln2mr", tag="ln2mr")
                nc.vector.tensor_mul(mrow[:], mu[:], rs[:])
                # broadcast rstd / mu*rstd on GPSIMD (SBUF-resident), then
                # split the normalize across DVE and GPSIMD
                rs_f = work_c.tile([P, TB], F32, name="rs_f", tag="rs_f")
                nc.gpsimd.partition_broadcast(rs_f[:], rs[:], channels=P)
                m_f = work_c.tile([P, TB], F32, name="m_f", tag="m_f")
                nc.gpsimd.partition_broadcast(m_f[:], mrow[:], channels=P)
                for c in range(CT):
                    if c < 5:
                        t1 = work_c.tile([P, TB], F32, name=f"xn2a{c}", tag="xn2a")
                        nc.vector.tensor_mul(t1[:], x2T[c][:], rs_f[:])
                        nc.vector.tensor_sub(x2m[c][:], t1[:], m_f[:])
                    else:
                        t1 = work_c.tile([P, TB], F32, name=f"xn2a{c}", tag="xn2b")
                        nc.gpsimd.tensor_mul(t1[:], x2T[c][:], rs_f[:])
                        nc.gpsimd.tensor_sub(x2m[c][:], t1[:], m_f[:])

            # ============ Phase D/E: fc1 + gelu, fc2 + residual ============
            HB = 4  # h-tiles per fc1 weight block
            with (
                tc.tile_pool(name="w1pool", bufs=1) as w1pool,
                tc.tile_pool(name="w2pool", bufs=1) as w2pool,
                tc.tile_pool(name="g1pool", bufs=1) as g1pool,
                tc.tile_pool(name="x3pool", bufs=1) as x3pool,
            ):
                g1 = [g1pool.tile([P, TB], MM_DT, name=f"g1_{h}", tag=f"g1_{h}")
                      for h in range(FF // P)]
                w2_all = w2pool.tile([P, (FF // P) * D], MM_DT, name="w2_all",
                                     tag="w2_all")
                w1t_l = [w1pool.tile([P, CT * HB * P], MM_DT, name=f"w1t{blk}",
                                     tag=f"w1t{blk % 3}")
                         for blk in range(FF // (HB * P))]
                # weight streams on the gpsimd queue: first two fc1 blocks,
                # then wfc2 chunks interleaved with the remaining fc1 blocks
                def load_w1(blk):
                    nc.scalar.dma_start(
                        w1t_l[blk][:].rearrange("p (c h) -> p c h", c=CT),
                        wfc1[:, blk * HB * P:(blk + 1) * HB * P]
                        .rearrange("(c p) h -> p c h", p=P))

                def load_w2(q):
                    nc.scalar.dma_start(
                        w2_all[:, q * 4 * D:(q + 1) * 4 * D]
                        .rearrange("p (ht j) -> p ht j", ht=4),
                        wfc2[q * 4 * P:(q + 1) * 4 * P, :]
                        .rearrange("(ht p) j -> p ht j", p=P))
                load_w1(0)
                load_w1(1)
                for blk in range(2, FF // (HB * P)):
                    for q in (2 * (blk - 2), 2 * (blk - 2) + 1):
                        if q < 8:
                            load_w2(q)
                    load_w1(blk)
                with tc.tile_pool(name="ps_fc1", bufs=2, space="PSUM") as ps_fc1:
                    for blk in range(FF // (HB * P)):
                        w1t = w1t_l[blk]
                        for hh in range(HB):
                            ht = blk * HB + hh
                            ps = ps_fc1.tile([P, TB], F32, name=f"fc1_{ht}",
                                             tag="fc1ps")
                            for c in range(CT):
                                nc.tensor.matmul(
                                    ps[:],
                                    w1t[:, c * HB * P + hh * P:
                                        c * HB * P + (hh + 1) * P],
                                    x2m[c][:],
                                    start=(c == 0), stop=(c == CT - 1))
                            nc.scalar.activation(g1[ht][:], ps[:], AF.Gelu,
                                                 bias=fc1B_sb[:, ht:ht + 1])

                with tc.tile_pool(name="ps_fc2", bufs=2, space="PSUM") as ps_fc2:
                    for c in range(CT):
                        ps = ps_fc2.tile([P, TB], F32, name=f"fc2_{c}", tag="fc2ps")
                        for ht in range(FF // P):
                            nc.tensor.matmul(
                                ps[:], w2_all[:, ht * D + c * P:ht * D + (c + 1) * P],
                                g1[ht][:],
                                start=(ht == 0), stop=(ht == FF // P - 1))
                        x3 = x3pool.tile([P, TB], F32, name=f"x3_{c}", tag="x3")
                        nc.vector.tensor_add(x3[:], ps[:], x2T[c][:])
                        nc.sync.dma_start(out_t[c * P:(c + 1) * P, :], x3[:])

    nc.compile()
    return nc


def _prep_inputs(x, ln1_w, ln1_b, w_qkv, w_proj, ln2_w, ln2_b, w_fc1, w_fc2):
    xf = np.ascontiguousarray(np.asarray(x, np.float32).reshape(TOKS, D))
    # fold ln1_w into wqkv rows; ln1_b contributes the rank-1 bqkv term
    ln1w_f = np.asarray(ln1_w, np.float32)
    ln1b_f = np.asarray(ln1_b, np.float32)
    wq = np.asarray(w_qkv[:, :D], np.float32) * 0.125  # fold 1/sqrt(dh)
    wk = np.asarray(w_qkv[:, D:2 * D], np.float32)
    wv = np.asarray(w_qkv[:, 2 * D:], np.float32)
    wqkv_f = np.concatenate([wq, wk, wv], axis=1)
    bqkv_full = ln1b_f @ wqkv_f
    bqkv_np = bqkv_full.reshape(1, 3 * D).astype(MM_NP)
    # per-partition column layout of the q,k parts for the ACT-copy bias
    bqkvc_np = np.ascontiguousarray(
        bqkv_full[:2 * D].reshape(2 * CT, P).T.astype(np.float32))
    wqkv_np = (ln1w_f[:, None] * wqkv_f).astype(MM_NP)
    wproj_np = np.asarray(w_proj, np.float32).astype(MM_NP)
    wfc2_np = np.asarray(w_fc2, np.float32).astype(MM_NP)
    # LN2 folded into fc1: scale wfc1 rows by ln2_w; B = ln2_b @ w_fc1;
    # A = -colsum of the bf16-rounded weights (matches the device matmul)
    w1p = (np.asarray(ln2_w, np.float32)[:, None] *
           np.asarray(w_fc1, np.float32))
    wfc1_np = w1p.astype(MM_NP)
    Bv = np.asarray(ln2_b, np.float32) @ np.asarray(w_fc1, np.float32)
    fc1B_np = np.ascontiguousarray(Bv.reshape(FF // P, P).T)
    pp, jj = np.meshgrid(np.arange(P), np.arange(P), indexing="ij")
    cm = np.where(pp <= jj, 0.0, -1e30).astype(np.float32)
    cmask2_np = np.ascontiguousarray(np.tile(cm, (1, 2)))
    common = {
        "wqkv": wqkv_np, "wproj": wproj_np, "wfc1": wfc1_np, "wfc2": wfc2_np,
        "fc1B": fc1B_np, "bqkv": bqkv_np, "bqkvc": bqkvc_np,
        "cmask2": cmask2_np,
        "id128m": np.eye(P, dtype=np.float32).astype(MM_NP),
    }
    in_maps = []
    for i in range(NC):
        m = dict(common)
        m["x_own"] = np.ascontiguousarray(xf[TB * i:TB * (i + 1)]).astype(MM_NP)
        in_maps.append(m)
    return in_maps


def _get_runner():
    """Build (once) a cached, non-donating PJRT executable for the kernel."""
    if "runner" in _cache:
        return _cache["runner"]
    import jax
    from jax.sharding import Mesh, PartitionSpec, NamedSharding
    from jax.experimental.shard_map import shard_map
    from concourse import bass2jax

    nc = _cache.get("nc")
    if nc is None:
        nc = _cache["nc"] = _build()
    bass2jax.install_neuronx_cc_hook()
    partition_name = nc.partition_id_tensor.name if nc.partition_id_tensor else None
    in_names, out_names, out_avals, zero_outs = [], [], [], []
    for alloc in nc.m.functions[0].allocations:
        if not isinstance(alloc, mybir.MemoryLocationSet):
            continue
        name = alloc.memorylocations[0].name
        if alloc.kind == "ExternalInput":
            if name != partition_name:
                in_names.append(name)
        elif alloc.kind == "ExternalOutput":
            out_names.append(name)
            shape = tuple(alloc.tensor_shape)
            dtype = mybir.dt.np(alloc.dtype)
            out_avals.append(jax.core.ShapedArray(shape, dtype))
            zero_outs.append(np.zeros(shape, dtype))
    n_params = len(in_names)
    all_in_names = in_names + out_names + ([partition_name] if partition_name else [])

    def _body(*args):
        operands = list(args)
        if partition_name is not None:
            operands.append(bass2jax.partition_id_tensor())
        outs = bass2jax._bass_exec_p.bind(
            *operands, out_avals=tuple(out_avals), in_names=tuple(all_in_names),
            out_names=tuple(out_names), lowering_input_output_aliases=(),
            sim_require_finite=True, sim_require_nnan=True, nc=nc)
        return tuple(outs)

    devices = jax.devices()[:NC]
    mesh = Mesh(np.asarray(devices), ("core",))
    nin = n_params + len(out_names)
    sharded = jax.jit(shard_map(
        _body, mesh=mesh, in_specs=(PartitionSpec("core"),) * nin,
        out_specs=(PartitionSpec("core"),) * len(out_names), check_rep=False))
    sh = NamedSharding(mesh, PartitionSpec("core"))
    dev_zeros = [
        jax.device_put(np.zeros((NC * z.shape[0], *z.shape[1:]), z.dtype), sh)
        for z in zero_outs
    ]
    runner = (sharded, in_names, out_names, out_avals, sh, dev_zeros)
    _cache["runner"] = runner
    return runner


def kernel(**inputs):
    import jax
    sharded, in_names, out_names, out_avals, sh, dev_zeros = _get_runner()
    in_maps = _prep_inputs(**inputs)
    concat_in = [np.concatenate([in_maps[c][nm] for c in range(NC)], axis=0)
                 for nm in in_names]
    dev_in = [jax.device_put(a, sh) for a in concat_in]
    out_arrs = sharded(*dev_in, *dev_zeros)
    got = {nm: np.asarray(out_arrs[i]).reshape(NC, *out_avals[i].shape)
           for i, nm in enumerate(out_names)}
    out = np.empty((TOKS, D), np.float32)
    for i in range(NC):
        out[TB * i:TB * (i + 1)] = got["out_t"][i].T
    return out.reshape(2, 2048, D)


if __name__ == "__main__":
    rng = np.random.default_rng(0)
    ins = {
        "x": rng.standard_normal((2, 2048, D), dtype=np.float32),
        "ln1_w": np.ones(D, np.float32),
        "ln1_b": np.zeros(D, np.float32),
        "w_qkv": (rng.standard_normal((D, 3 * D), dtype=np.float32) / 32.0),
        "w_proj": (rng.standard_normal((D, D), dtype=np.float32) / 32.0),
        "ln2_w": np.ones(D, np.float32),
        "ln2_b": np.zeros(D, np.float32),
        "w_fc1": (rng.standard_normal((D, FF), dtype=np.float32) / 32.0),
        "w_fc2": (rng.standard_normal((FF, D), dtype=np.float32) / 64.0),
    }
    out = kernel(**ins)
    print("kernel out", out.shape, out.dtype, float(np.abs(out).mean()))


# revision 54
# speedup vs baseline: 1.3047x; 1.3047x over previous
"""Trainium2 Bass kernel for a dense transformer block (B=2, T=2048, D=1024, H=16).

Sharding (8 NeuronCores, one chip):
  - Token-split everywhere except attention: core i owns 512 tokens (rows
    512i:512i+512 of the flattened [4096, 1024] activation).
  - Head-split attention: core i owns heads {2i, 2i+1}.
  - Collectives (all AllToAll; no AllGather since AG is charged on its 8x
    output):
      #1  Q,K   (each core computes QKV for its own tokens, all heads, then
                 redistributes per-head)    [8, 256, 512] bf16
      #2  V     (token-major per k-tile)    [8, 512, 128] bf16
      #3  attention outputs back to token owners  [8, 128, 512] bf16
    #2 overlaps score compute; #1 overlaps V compute + residual transposes.

Layout is feature-major ("transposed", [feature, token]) on-chip; transposes
run on the PE (idle during the front-end) with DVE/ACT copies out of PSUM.
LN1 is folded into wqkv (w) and per-partition biases (b); LN2 is applied to
the residual stream BEFORE fc1 (w folded into wfc1, b as gelu bias), so the
MLP needs no per-h-tile unfold work on DVE. DMAs are few and large, spread
across the SP/ACT/Pool queues by dependency class (each queue sustains only
~1 transfer per ~3.6us in the cost model, and a DMA waiting on a collective
blocks everything behind it on the same queue).
"""

import numpy as np
import ml_dtypes

import concourse.bass as bass
import concourse.mybir as mybir
import concourse.tile as tile
from concourse import bacc
from concourse import bass_utils

F32 = mybir.dt.float32
MM_DT = mybir.dt.bfloat16
MM_NP = ml_dtypes.bfloat16

P = 128            # partitions
TB = 512           # tokens per core
D = 1024           # model dim
CT = D // P        # 8 feature tiles
NC = 8             # cores
TOKS = 2 * 2048    # global tokens
FF = 4096          # mlp hidden
DH = 64            # head dim
VW = 2 * (DH + 1)  # per-k-tile V columns: 2 heads x (64 dims + 1 ones col)
NKT = 32           # global k-tiles of 128 tokens

AF = mybir.ActivationFunctionType
OP = mybir.AluOpType
RG = [list(range(NC))]

ET_BUFS = 52       # et lookahead window (scores run ahead of AV by this much)

_cache: dict = {}


def _build():
    nc = bacc.Bacc(
        "TRN2",
        target_bir_lowering=False,
        debug=False,
        enable_asserts=False,
        num_devices=NC,
    )

    # ---- kernel I/O ----
    x_own = nc.dram_tensor("x_own", [TB, D], MM_DT, kind="ExternalInput").ap()
    wqkv = nc.dram_tensor("wqkv", [D, 3 * D], MM_DT, kind="ExternalInput").ap()
    wproj = nc.dram_tensor("wproj", [D, D], MM_DT, kind="ExternalInput").ap()
    wfc1 = nc.dram_tensor("wfc1", [D, FF], MM_DT, kind="ExternalInput").ap()
    wfc2 = nc.dram_tensor("wfc2", [FF, D], MM_DT, kind="ExternalInput").ap()
    fc1B = nc.dram_tensor("fc1B", [P, FF // P], F32, kind="ExternalInput").ap()
    bqkv = nc.dram_tensor("bqkv", [1, 3 * D], MM_DT, kind="ExternalInput").ap()
    bqkvc = nc.dram_tensor("bqkvc", [P, 2 * CT], F32, kind="ExternalInput").ap()
    cmask2 = nc.dram_tensor("cmask2", [P, 2 * P], F32, kind="ExternalInput").ap()
    id128m = nc.dram_tensor("id128m", [P, P], MM_DT, kind="ExternalInput").ap()
    out_t = nc.dram_tensor("out_t", [D, TB], F32, kind="ExternalOutput").ap()

    with tile.TileContext(nc) as tc:
        with (
            tc.tile_pool(name="persist", bufs=1) as pers,
            tc.tile_pool(name="dram", bufs=1, space="DRAM") as dram,
        ):
            # collective buffers
            qk_in = dram.tile([NC, 2 * P, TB], MM_DT, name="qk_in", tag="qk_in")
            qk_out = dram.tile([NC, 2 * P, TB], MM_DT, name="qk_out", tag="qk_out")
            v_in = dram.tile([NC, TB, P], MM_DT, name="v_in", tag="v_in")
            v_out = dram.tile([NC, TB, P], MM_DT, name="v_out", tag="v_out")
            y_in = dram.tile([NC, P, TB], MM_DT, name="y_in", tag="y_in")
            y_out = dram.tile([NC, P, TB], MM_DT, name="y_out", tag="y_out")

            # small constants
            ones_col_m = pers.tile([P, 1], MM_DT, name="ones_col_m", tag="ones_col_m")
            nc.gpsimd.memset(ones_col_m[:], 1.0)
            ones_row_m = pers.tile([1, DH], MM_DT, name="ones_row_m", tag="ones_row_m")
            nc.gpsimd.memset(ones_row_m[:], 1.0)
            ones_row_f = pers.tile([1, P], F32, name="ones_row_f", tag="ones_row_f")
            nc.gpsimd.memset(ones_row_f[:], 1.0)
            zb = pers.tile([P, 1], F32, name="zb", tag="zb")
            nc.gpsimd.memset(zb[:], 0.0)
            eps1 = pers.tile([1, 1], F32, name="eps1", tag="eps1")
            nc.gpsimd.memset(eps1[:], 1e-5)
            epsP = pers.tile([P, 1], F32, name="epsP", tag="epsP")
            nc.gpsimd.memset(epsP[:], 1e-5)
            cmask_sb = pers.tile([P, 2 * P], F32, name="cmask_sb", tag="cmask_sb")
            nc.gpsimd.dma_start(cmask_sb[:], cmask2[:])
            idm_sb = pers.tile([P, P], MM_DT, name="idm_sb", tag="idm_sb")
            nc.gpsimd.dma_start(idm_sb[:], id128m[:])
            fc1B_sb = pers.tile([P, FF // P], F32, name="fc1B_sb", tag="fc1B_sb")
            bqkvc_sb = pers.tile([P, 2 * CT], F32, name="bqkvc_sb", tag="bqkvc_sb")

            # persistent activations
            xT = pers.tile([P, CT * TB], F32, name="xT", tag="xT")  # raw x, feat-major
            x2T = [pers.tile([P, TB], F32, name=f"x2T{c}", tag=f"x2T{c}")
                   for c in range(CT)]
            qT = pers.tile([P, TOKS], MM_DT, name="qT", tag="qT")
            kT = pers.tile([P, TOKS], MM_DT, name="kT", tag="kT")
            V_sb = pers.tile([P, NKT * VW], MM_DT, name="V_sb", tag="V_sb")
            # ones columns (data columns DMA'd in later)
            nc.gpsimd.memset(
                V_sb[:].rearrange("p (g w) -> p g w", w=DH + 1)[:, :, DH:DH + 1], 1.0)
            x2m = [pers.tile([P, TB], MM_DT, name=f"x2m{c}", tag=f"x2m{c}")
                   for c in range(CT)]
            wproj_sb = pers.tile([P, CT * D], MM_DT, name="wproj_sb", tag="wproj_sb")

            # ============ Phase A: load, LN1, QKV, A2A #1/#2 ============
            with (
                tc.tile_pool(name="ps_a", bufs=6, space="PSUM") as ps_a,
                tc.tile_pool(name="ps_t", bufs=2, space="PSUM") as ps_t,
                tc.tile_pool(name="work_a", bufs=2) as work_a,
                tc.tile_pool(name="xrow_a", bufs=1) as xrow_a,
                tc.tile_pool(name="wqkv_a", bufs=1) as wqkv_a,
                tc.tile_pool(name="xn_a", bufs=1) as xn_a,
                tc.tile_pool(name="qkl_a", bufs=1) as qkl_a,
                tc.tile_pool(name="vl_a", bufs=1) as vl_a,
            ):
                # single big DMAs: per-queue throughput is the scarce resource
                bqkv_sb = xrow_a.tile([1, 3 * D], MM_DT, name="bqkv_sb",
                                      tag="bqkv_sb")
                ones_row_p = xrow_a.tile([1, P], MM_DT, name="ones_row_p",
                                         tag="ones_row_p")
                nc.gpsimd.memset(ones_row_p[:], 1.0)
                xrow = xrow_a.tile([P, 4 * D], MM_DT, name="xrow", tag="xrow")
                nc.sync.dma_start(
                    xrow[:].rearrange("p (r c) -> p r c", r=4),
                    x_own[:].rearrange("(r p) c -> p r c", p=P))
                wqkv_sb = wqkv_a.tile([P, CT * 3 * D], MM_DT, name="wqkv_sb",
                                      tag="wqkv_sb")

                def wq(c):  # [128, 3*D] block of k-tile c
                    return wqkv_sb[:, c * 3 * D:(c + 1) * 3 * D]

                for half in range(2):
                    nc.sync.dma_start(
                        wqkv_sb[:, half * 4 * 3 * D:(half + 1) * 4 * 3 * D]
                        .rearrange("p (c j) -> p c j", c=4),
                        wqkv[half * 4 * P:(half + 1) * 4 * P, :]
                        .rearrange("(c p) j -> p c j", p=P))
                nc.scalar.dma_start(bqkv_sb[:], bqkv[:])
                nc.scalar.dma_start(bqkvc_sb[:], bqkvc[:])
                nc.gpsimd.dma_start(fc1B_sb[:], fc1B[:])

                # LN1 per row-block (natural layout, bf16; ln1_w folded into
                # wqkv, ln1_b via bias terms). DVE-only chain except the Sqrt.
                xnn_l = [xn_a.tile([P, D], MM_DT, name=f"xnn{r}", tag=f"xnn{r}")
                         for r in range(TB // P)]
                for r in range(TB // P):
                    xr = xrow[:, r * D:(r + 1) * D]
                    s1 = work_a.tile([P, 1], F32, name=f"s1_{r}", tag="lns1")
                    nc.vector.tensor_reduce(s1[:], xr,
                                            axis=mybir.AxisListType.X, op=OP.add)
                    sqf = work_a.tile([P, D], MM_DT, name=f"sq_{r}", tag="lnsq")
                    s2 = work_a.tile([P, 1], F32, name=f"s2_{r}", tag="lns2")
                    nc.vector.scalar_tensor_tensor(
                        out=sqf[:], in0=xr, scalar=1.0, in1=xr,
                        op0=OP.mult, op1=OP.mult, accum_out=s2[:])
                    mu = work_a.tile([P, 1], F32, name=f"mu_{r}", tag="lnmu")
                    nc.vector.tensor_scalar_mul(mu[:], s1[:], 1.0 / D)
                    mu2 = work_a.tile([P, 1], F32, name=f"mu2_{r}", tag="lnmu2")
                    nc.vector.tensor_mul(mu2[:], mu[:], mu[:])
                    var = work_a.tile([P, 1], F32, name=f"var_{r}", tag="lnvar")
                    nc.vector.scalar_tensor_tensor(
                        out=var[:], in0=s2[:], scalar=1.0 / D, in1=mu2[:],
                        op0=OP.mult, op1=OP.subtract)
                    sd = work_a.tile([P, 1], F32, name=f"sd_{r}", tag="lnsd")
                    nc.scalar.activation(sd[:], var[:], AF.Sqrt, bias=epsP[:])
                    rstd = work_a.tile([P, 1], F32, name=f"rstd_{r}", tag="lnrstd")
                    nc.vector.reciprocal(rstd[:], sd[:])
                    nc.vector.tensor_scalar(xnn_l[r][:], xr, mu[:], rstd[:],
                                            OP.subtract, OP.mult)

                # transposes on the (idle) PE; copies split across DVE/ACT
                xnT = xn_a.tile([P, CT * TB], MM_DT, name="xnT", tag="xnT")
                for r in range(TB // P):
                    for c in range(CT):
                        pt = ps_t.tile([P, P], MM_DT, name=f"ptn_{r}_{c}",
                                       tag="ptn")
                        nc.tensor.transpose(
                            pt[:], xnn_l[r][:, c * P:(c + 1) * P], idm_sb[:])
                        nc.vector.tensor_copy(
                            xnT[:, c * TB + r * P:c * TB + (r + 1) * P], pt[:])

                # ---- Q,K matmuls (feature-major, my 512 tokens) ----
                # c-major in groups so the PE streams while later weight
                # chunks are still in flight
                qkl = qkl_a.tile([P, NC * 2 * TB], MM_DT, name="qkl", tag="qkl")
                for g in range(2):  # 0=q, 1=k
                    ps_l = [ps_a.tile([P, TB], F32, name=f"qk{d}_{g}", tag="psqk")
                            for d in range(NC)]
                    for c in range(CT):
                        for d in range(NC):
                            nc.tensor.matmul(
                                ps_l[d][:], wq(c)[:, g * D + d * P:
                                                  g * D + (d + 1) * P],
                                xnT[:, c * TB:(c + 1) * TB],
                                start=(c == 0), stop=(c == CT - 1))
                    for d in range(NC):
                        # ln1_b contribution (b @ W) as per-partition bias
                        nc.scalar.activation(
                            qkl[:, (2 * d + g) * TB:(2 * d + g + 1) * TB],
                            ps_l[d][:], AF.Identity,
                            bias=bqkvc_sb[:, g * CT + d:g * CT + d + 1])
                    if g == 0:
                        nc.scalar.dma_start(
                            qk_in[:, 0:P, :].rearrange("d p t -> p d t"),
                            qkl[:].rearrange("p (d gg t) -> p d gg t", gg=2, t=TB)
                            [:, :, 0, :])
                    else:
                        nc.sync.dma_start(
                            qk_in[:, P:2 * P, :].rearrange("d p t -> p d t"),
                            qkl[:].rearrange("p (d gg t) -> p d gg t", gg=2, t=TB)
                            [:, :, 1, :])
                nc.gpsimd.collective_compute(
                    "AllToAll", OP.bypass, replica_groups=RG,
                    ins=[qk_in[:]], outs=[qk_out[:]])

                # ---- V matmuls (token-major) + A2A #2 ----
                vloc = vl_a.tile([P, 4 * D], MM_DT, name="vloc", tag="vloc")
                for tt in range(TB // P):
                    for nh in range(2):
                        ps = ps_a.tile([P, TB], F32, name=f"v{tt}_{nh}", tag="psqk")
                        nc.tensor.matmul(
                            ps[:], ones_row_p[:],
                            bqkv_sb[0:1, 2 * D + nh * TB:2 * D + (nh + 1) * TB],
                            start=True, stop=False)
                        for c in range(CT):
                            nc.tensor.matmul(
                                ps[:],
                                xnT[:, c * TB + tt * P:c * TB + (tt + 1) * P],
                                wq(c)[:, 2 * D + nh * TB:2 * D + (nh + 1) * TB],
                                start=False, stop=(c == CT - 1))
                        # write dest-major: vloc[p, (d*4 + tt)*P + c]
                        nc.scalar.activation(
                            vloc[:].rearrange("p (q tt c) -> p q tt c",
                                              tt=4, c=P)
                            [:, nh * 4:(nh + 1) * 4, tt, :],
                            ps[:], AF.Copy)
                nc.sync.dma_start(
                    v_in[:].rearrange("d (tt p) c -> p (d tt) c", p=P),
                    vloc[:].rearrange("p (dt c) -> p dt c", c=P))
                nc.gpsimd.collective_compute(
                    "AllToAll", OP.bypass, replica_groups=RG,
                    ins=[v_in[:]], outs=[v_out[:]])

                # ---- residual-path transposes of raw x (fills A2A wait) ----
                for r in range(TB // P):
                    for c in range(CT):
                        pt = ps_t.tile([P, P], MM_DT, name=f"ptr_{r}_{c}",
                                       tag="ptn")
                        nc.tensor.transpose(
                            pt[:], xrow[:, r * D + c * P:r * D + (c + 1) * P],
                            idm_sb[:])
                        nc.scalar.activation(
                            xT[:, c * TB + r * P:c * TB + (r + 1) * P],
                            pt[:], AF.Copy)
                for half in range(2):
                    nc.scalar.dma_start(
                        wproj_sb[:, half * 4 * D:(half + 1) * 4 * D]
                        .rearrange("p (c j) -> p c j", c=4),
                        wproj[half * 4 * P:(half + 1) * 4 * P, :]
                        .rearrange("(c p) j -> p c j", p=P))

            # ============ Phase B: attention ============
            with (
                tc.tile_pool(name="ps_s", bufs=2, space="PSUM") as ps_s_pool,
                tc.tile_pool(name="ps_y", bufs=2, space="PSUM") as ps_y_pool,
                tc.tile_pool(name="epool", bufs=ET_BUFS) as epool,
                tc.tile_pool(name="work_b", bufs=4) as work_b,
                tc.tile_pool(name="rb_b", bufs=2) as rb_b,
            ):
                # receive q/k (feature-major, all tokens for my 2 heads);
                # batch-0 sources first so b=0 scores start immediately
                for half in range(2):
                    sl = slice(half * 4, (half + 1) * 4)
                    nc.sync.dma_start(
                        kT[:, half * 4 * TB:(half + 1) * 4 * TB]
                        .rearrange("p (s t) -> p s t", s=4),
                        qk_out[sl, P:2 * P, :].rearrange("s p t -> p s t"))
                    nc.scalar.dma_start(
                        qT[:, half * 4 * TB:(half + 1) * 4 * TB]
                        .rearrange("p (s t) -> p s t", s=4),
                        qk_out[sl, 0:P, :].rearrange("s p t -> p s t"))
                # receive V into packed [tok, (head, dh|one)] slots
                for h in range(2):
                    nc.sync.dma_start(
                        V_sb[:].rearrange("p (gk hh w) -> p gk hh w",
                                          w=DH + 1, hh=2)[:, :, h, 0:DH],
                        v_out[:].rearrange("s (tt p) (hh c) -> p (s tt) hh c",
                                           p=P, c=DH)[:, :, h, :])

                # score/exp jobs and AV jobs, interleaved with a lookahead
                # window so the PE never sits behind a V-gated AV while
                # score work remains.
                sjobs = []   # (b, j, kt, pss, et)
                for b in range(2):
                    for j in range(4):
                        for kt in range(4 * j + 4):
                            sjobs.append([b, j, kt, None, None])

                def emit_score(job):
                    b, j, kt, _, _ = job
                    jg = b * 4 + j
                    qoff = jg * TB
                    gk = b * 16 + kt
                    n0 = 0 if kt < 4 * j else (kt - 4 * j) * P
                    pss = ps_s_pool.tile([P, 2 * TB], F32,
                                         name=f"pss{b}_{j}_{kt}", tag="pss")
                    for h in range(2):
                        nc.tensor.matmul(
                            pss[:, h * TB + n0:(h + 1) * TB],
                            kT[h * DH:(h + 1) * DH, gk * P:(gk + 1) * P],
                            qT[h * DH:(h + 1) * DH, qoff + n0:qoff + TB],
                            start=True, stop=True,
                            tile_position=(h * DH, 0))
                    if kt >= 4 * j:
                        nc.vector.tensor_add(
                            pss[:].rearrange("p (h t) -> p h t", h=2)
                            [:, :, n0:n0 + P],
                            pss[:].rearrange("p (h t) -> p h t", h=2)
                            [:, :, n0:n0 + P],
                            cmask_sb[:].rearrange("p (h t) -> p h t", h=2))
                    et = epool.tile([P, 2 * TB], MM_DT,
                                    name=f"et{b}_{j}_{kt}", tag="et")
                    if n0 == 0:
                        nc.scalar.activation(et[:], pss[:], AF.Exp, bias=zb[:])
                    else:
                        nc.scalar.activation(
                            et[:].rearrange("p (h t) -> p h t", h=2)[:, :, n0:TB],
                            pss[:].rearrange("p (h t) -> p h t", h=2)[:, :, n0:TB],
                            AF.Exp, bias=zb[:])
                    job[3] = pss
                    job[4] = et

                si = 0
                for si in range(min(ET_BUFS - 2, len(sjobs))):
                    emit_score(sjobs[si])
                si += 1

                for b in range(2):
                    for j in range(4):
                        jg = b * 4 + j
                        nkt = 4 * j + 4
                        ps_y = [ps_y_pool.tile([DH + 1, TB], F32,
                                               name=f"psy{b}_{j}_{h}", tag=f"psy{h}")
                                for h in range(2)]
                        base = sum(4 * (g % 4) + 4 for g in range(4 * b + j))
                        for kt in range(nkt):
                            job = sjobs[base + kt]
                            _, _, _, pss, et = job
                            n0 = 0 if kt < 4 * j else (kt - 4 * j) * P
                            gk = b * 16 + kt
                            for h in range(2):
                                vsl = V_sb[:, gk * VW + h * (DH + 1):
                                           gk * VW + h * (DH + 1) + DH + 1]
                                nc.tensor.matmul(
                                    ps_y[h][:, n0:TB], vsl,
                                    et[:, h * TB + n0:(h + 1) * TB],
                                    start=(kt == 0), stop=(kt == nkt - 1))
                            if si < len(sjobs):
                                emit_score(sjobs[si])
                                si += 1
                        # normalize: y / colsum -> bf16 -> a2a staging
                        yn = work_b.tile([P, TB], MM_DT, name=f"yn{b}_{j}",
                                         tag="yn")
                        for h in range(2):
                            rf = work_b.tile([1, TB], F32, name=f"rf{b}_{j}_{h}",
                                             tag="rf")
                            nc.vector.reciprocal(rf[:], ps_y[h][DH:DH + 1, :])
                            # denominator broadcast on the idle GPSIMD
                            rb = rb_b.tile([DH, TB], F32, name=f"rb{b}_{j}_{h}",
                                           tag="rb")
                            nc.gpsimd.partition_broadcast(rb[:], rf[:],
                                                          channels=DH)
                            nc.vector.tensor_mul(
                                yn[h * DH:(h + 1) * DH, :],
                                ps_y[h][0:DH, :], rb[:])
                        nc.sync.dma_start(y_in[jg], yn[:])

                nc.gpsimd.collective_compute(
                    "AllToAll", OP.bypass, replica_groups=RG,
                    ins=[y_in[:]], outs=[y_out[:]])

            # ============ Phase C: proj + residual + LN2 ============
            with (
                tc.tile_pool(name="ps_c", bufs=2, space="PSUM") as ps_c,
                tc.tile_pool(name="ps_ln", bufs=2, space="PSUM") as ps_ln,
                tc.tile_pool(name="ps_bc", bufs=2, space="PSUM") as ps_bc,
                tc.tile_pool(name="work_c", bufs=2) as work_c,
                tc.tile_pool(name="yall_c", bufs=1) as yall_c,
            ):
                yall = yall_c.tile([P, NC * TB], MM_DT, name="yall", tag="yall")
                nc.scalar.dma_start(
                    yall[:, 0:4 * TB].rearrange("p (s t) -> p s t", s=4),
                    y_out[0:4].rearrange("s p t -> p s t"))
                nc.sync.dma_start(
                    yall[:, 4 * TB:].rearrange("p (s t) -> p s t", s=4),
                    y_out[4:8].rearrange("s p t -> p s t"))
                s1 = ps_ln.tile([1, TB], F32, name="ln2s1", tag="lnsum")
                s2 = ps_ln.tile([1, TB], F32, name="ln2s2", tag="lnsum")
                for c in range(CT):
                    ps = ps_c.tile([P, TB], F32, name=f"proj{c}", tag="projps")
                    for s in range(NC):
                        nc.tensor.matmul(
                            ps[:],
                            wproj_sb[:, s * D + c * P:s * D + (c + 1) * P],
                            yall[:, s * TB:(s + 1) * TB],
                            start=(s == 0), stop=(s == NC - 1))
                    nc.vector.tensor_add(x2T[c][:], ps[:],
                                         xT[:, c * TB:(c + 1) * TB])
                    # LN2 stats via ones-matmuls on a bf16 scratch copy
                    x2s = work_c.tile([P, TB], MM_DT, name=f"x2s{c}", tag="x2s")
                    nc.scalar.activation(x2s[:], x2T[c][:], AF.Copy)
                    nc.tensor.matmul(s1[:], ones_col_m[:], x2s[:],
                                     start=(c == 0), stop=(c == CT - 1))
                    sq = work_c.tile([P, TB], MM_DT, name=f"sq{c}", tag="ln2sq")
                    nc.vector.tensor_mul(sq[:], x2s[:], x2s[:])
                    nc.tensor.matmul(s2[:], ones_col_m[:], sq[:],
                                     start=(c == 0), stop=(c == CT - 1))
                mu = work_c.tile([1, TB], F32, name="ln2mu", tag="ln2mu")
                nc.vector.tensor_scalar_mul(mu[:], s1[:], 1.0 / D)
                mu2 = work_c.tile([1, TB], F32, name="ln2mu2", tag="ln2mu2")
                nc.vector.tensor_mul(mu2[:], mu[:], mu[:])
                vr = work_c.tile([1, TB], F32, name="ln2vr", tag="ln2vr")
                nc.vector.scalar_tensor_tensor(
                    out=vr[:], in0=s2[:], scalar=1.0 / D, in1=mu2[:],
                    op0=OP.mult, op1=OP.subtract)
                sdr = work_c.tile([1, TB], F32, name="ln2sd", tag="ln2sd")
                nc.scalar.activation(sdr[:], vr[:], AF.Sqrt, bias=eps1[:])
                rs = work_c.tile([1, TB], F32, name="ln2rs", tag="ln2rs")
                nc.vector.reciprocal(rs[:], sdr[:])
                mrow = work_c.tile([1, TB], F32, name="ln2mr", tag="ln2mr")
                nc.vector.tensor_mul(mrow[:], mu[:], rs[:])
                # broadcast rstd / mu*rstd on GPSIMD (SBUF-resident), then
                # split the normalize across DVE and GPSIMD
                rs_f = work_c.tile([P, TB], F32, name="rs_f", tag="rs_f")
                nc.gpsimd.partition_broadcast(rs_f[:], rs[:], channels=P)
                m_f = work_c.tile([P, TB], F32, name="m_f", tag="m_f")
                nc.gpsimd.partition_broadcast(m_f[:], mrow[:], channels=P)
                for c in range(CT):
                    if c < 5:
                        t1 = work_c.tile([P, TB], F32, name=f"xn2a{c}", tag="xn2a")
                        nc.vector.tensor_mul(t1[:], x2T[c][:], rs_f[:])
                        nc.vector.tensor_sub(x2m[c][:], t1[:], m_f[:])
                    else:
                        t1 = work_c.tile([P, TB], F32, name=f"xn2a{c}", tag="xn2b")
                        nc.gpsimd.tensor_mul(t1[:], x2T[c][:], rs_f[:])
                        nc.gpsimd.tensor_sub(x2m[c][:], t1[:], m_f[:])

            # ============ Phase D/E: fc1 + gelu, fc2 + residual ============
            HB = 4  # h-tiles per fc1 weight block
            with (
                tc.tile_pool(name="w1pool", bufs=1) as w1pool,
                tc.tile_pool(name="w2pool", bufs=1) as w2pool,
                tc.tile_pool(name="g1pool", bufs=1) as g1pool,
                tc.tile_pool(name="x3pool", bufs=1) as x3pool,
            ):
                g1 = [g1pool.tile([P, TB], MM_DT, name=f"g1_{h}", tag=f"g1_{h}")
                      for h in range(FF // P)]
                w2_all = w2pool.tile([P, (FF // P) * D], MM_DT, name="w2_all",
                                     tag="w2_all")
                w1t_l = [w1pool.tile([P, CT * HB * P], MM_DT, name=f"w1t{blk}",
                                     tag=f"w1t{blk % 3}")
                         for blk in range(FF // (HB * P))]
                # weight streams on the gpsimd queue: first two fc1 blocks,
                # then wfc2 chunks interleaved with the remaining fc1 blocks
                def load_w1(blk):
                    nc.scalar.dma_start(
                        w1t_l[blk][:].rearrange("p (c h) -> p c h", c=CT),
                        wfc1[:, blk * HB * P:(blk + 1) * HB * P]
                        .rearrange("(c p) h -> p c h", p=P))

                def load_w2(q):
                    nc.scalar.dma_start(
                        w2_all[:, q * 4 * D:(q + 1) * 4 * D]
                        .rearrange("p (ht j) -> p ht j", ht=4),
                        wfc2[q * 4 * P:(q + 1) * 4 * P, :]
                        .rearrange("(ht p) j -> p ht j", p=P))
                load_w1(0)
                load_w1(1)
                for blk in range(2, FF // (HB * P)):
                    for q in (2 * (blk - 2), 2 * (blk - 2) + 1):
                        if q < 8:
                            load_w2(q)
                    load_w1(blk)
                with tc.tile_pool(name="ps_fc1", bufs=2, space="PSUM") as ps_fc1:
                    for blk in range(FF // (HB * P)):
                        w1t = w1t_l[blk]
                        for hh in range(HB):
                            ht = blk * HB + hh
                            ps = ps_fc1.tile([P, TB], F32, name=f"fc1_{ht}",
                                             tag="fc1ps")
                            for c in range(CT):
                                nc.tensor.matmul(
                                    ps[:],
                                    w1t[:, c * HB * P + hh * P:
                                        c * HB * P + (hh + 1) * P],
                                    x2m[c][:],
                                    start=(c == 0), stop=(c == CT - 1))
                            nc.scalar.activation(g1[ht][:], ps[:], AF.Gelu,
                                                 bias=fc1B_sb[:, ht:ht + 1])

                with tc.tile_pool(name="ps_fc2", bufs=2, space="PSUM") as ps_fc2:
                    for c in range(CT):
                        ps = ps_fc2.tile([P, TB], F32, name=f"fc2_{c}", tag="fc2ps")
                        for ht in range(FF // P):
                            nc.tensor.matmul(
                                ps[:], w2_all[:, ht * D + c * P:ht * D + (c + 1) * P],
                                g1[ht][:],
                                start=(ht == 0), stop=(ht == FF // P - 1))
                        x3 = x3pool.tile([P, TB], F32, name=f"x3_{c}", tag="x3")
                        nc.vector.tensor_add(x3[:], ps[:], x2T[c][:])
                        nc.sync.dma_start(out_t[c * P:(c + 1) * P, :], x3[:])

    nc.compile()
    return nc


def _prep_inputs(x, ln1_w, ln1_b, w_qkv, w_proj, ln2_w, ln2_b, w_fc1, w_fc2):
    xf = np.ascontiguousarray(np.asarray(x, np.float32).reshape(TOKS, D))
    # fold ln1_w into wqkv rows; ln1_b contributes the rank-1 bqkv term
    ln1w_f = np.asarray(ln1_w, np.float32)
    ln1b_f = np.asarray(ln1_b, np.float32)
    wq = np.asarray(w_qkv[:, :D], np.float32) * 0.125  # fold 1/sqrt(dh)
    wk = np.asarray(w_qkv[:, D:2 * D], np.float32)
    wv = np.asarray(w_qkv[:, 2 * D:], np.float32)
    wqkv_f = np.concatenate([wq, wk, wv], axis=1)
    bqkv_full = ln1b_f @ wqkv_f
    bqkv_np = bqkv_full.reshape(1, 3 * D).astype(MM_NP)
    # per-partition column layout of the q,k parts for the ACT-copy bias
    bqkvc_np = np.ascontiguousarray(
        bqkv_full[:2 * D].reshape(2 * CT, P).T.astype(np.float32))
    wqkv_np = (ln1w_f[:, None] * wqkv_f).astype(MM_NP)
    wproj_np = np.asarray(w_proj, np.float32).astype(MM_NP)
    wfc2_np = np.asarray(w_fc2, np.float32).astype(MM_NP)
    # LN2 folded into fc1: scale wfc1 rows by ln2_w; B = ln2_b @ w_fc1;
    # A = -colsum of the bf16-rounded weights (matches the device matmul)
    w1p = (np.asarray(ln2_w, np.float32)[:, None] *
           np.asarray(w_fc1, np.float32))
    wfc1_np = w1p.astype(MM_NP)
    Bv = np.asarray(ln2_b, np.float32) @ np.asarray(w_fc1, np.float32)
    fc1B_np = np.ascontiguousarray(Bv.reshape(FF // P, P).T)
    pp, jj = np.meshgrid(np.arange(P), np.arange(P), indexing="ij")
    cm = np.where(pp <= jj, 0.0, -1e30).astype(np.float32)
    cmask2_np = np.ascontiguousarray(np.tile(cm, (1, 2)))
    common = {
        "wqkv": wqkv_np, "wproj": wproj_np, "wfc1": wfc1_np, "wfc2": wfc2_np,
        "fc1B": fc1B_np, "bqkv": bqkv_np, "bqkvc": bqkvc_np,
        "cmask2": cmask2_np,
        "id128m": np.eye(P, dtype=np.float32).astype(MM_NP),
    }
    in_maps = []
    for i in range(NC):
        m = dict(common)
        m["x_own"] = np.ascontiguousarray(xf[TB * i:TB * (i + 1)]).astype(MM_NP)
        in_maps.append(m)
    return in_maps


def _get_runner():
    """Build (once) a cached, non-donating PJRT executable for the kernel."""
    if "runner" in _cache:
        return _cache["runner"]
    import jax
    from jax.sharding import Mesh, PartitionSpec, NamedSharding
    from jax.experimental.shard_map import shard_map
    from concourse import bass2jax

    nc = _cache.get("nc")
    if nc is None:
        nc = _cache["nc"] = _build()
    bass2jax.install_neuronx_cc_hook()
    partition_name = nc.partition_id_tensor.name if nc.partition_id_tensor else None
    in_names, out_names, out_avals, zero_outs = [], [], [], []
    for alloc in nc.m.functions[0].allocations:
        if not isinstance(alloc, mybir.MemoryLocationSet):
            continue
        name = alloc.memorylocations[0].name
        if alloc.kind == "ExternalInput":
            if name != partition_name:
                in_names.append(name)
        elif alloc.kind == "ExternalOutput":
            out_names.append(name)
            shape = tuple(alloc.tensor_shape)
            dtype = mybir.dt.np(alloc.dtype)
            out_avals.append(jax.core.ShapedArray(shape, dtype))
            zero_outs.append(np.zeros(shape, dtype))
    n_params = len(in_names)
    all_in_names = in_names + out_names + ([partition_name] if partition_name else [])

    def _body(*args):
        operands = list(args)
        if partition_name is not None:
            operands.append(bass2jax.partition_id_tensor())
        outs = bass2jax._bass_exec_p.bind(
            *operands, out_avals=tuple(out_avals), in_names=tuple(all_in_names),
            out_names=tuple(out_names), lowering_input_output_aliases=(),
            sim_require_finite=True, sim_require_nnan=True, nc=nc)
        return tuple(outs)

    devices = jax.devices()[:NC]
    mesh = Mesh(np.asarray(devices), ("core",))
    nin = n_params + len(out_names)
    sharded = jax.jit(shard_map(
        _body, mesh=mesh, in_specs=(PartitionSpec("core"),) * nin,
        out_specs=(PartitionSpec("core"),) * len(out_names), check_rep=False))
    sh = NamedSharding(mesh, PartitionSpec("core"))
    dev_zeros = [
        jax.device_put(np.zeros((NC * z.shape[0], *z.shape[1:]), z.dtype), sh)
        for z in zero_outs
    ]
    runner = (sharded, in_names, out_names, out_avals, sh, dev_zeros)
    _cache["runner"] = runner
    return runner


def kernel(**inputs):
    import jax
    sharded, in_names, out_names, out_avals, sh, dev_zeros = _get_runner()
    in_maps = _prep_inputs(**inputs)
    concat_in = [np.concatenate([in_maps[c][nm] for c in range(NC)], axis=0)
                 for nm in in_names]
    dev_in = [jax.device_put(a, sh) for a in concat_in]
    out_arrs = sharded(*dev_in, *dev_zeros)
    got = {nm: np.asarray(out_arrs[i]).reshape(NC, *out_avals[i].shape)
           for i, nm in enumerate(out_names)}
    out = np.empty((TOKS, D), np.float32)
    for i in range(NC):
        out[TB * i:TB * (i + 1)] = got["out_t"][i].T
    return out.reshape(2, 2048, D)


if __name__ == "__main__":
    rng = np.random.default_rng(0)
    ins = {
        "x": rng.standard_normal((2, 2048, D), dtype=np.float32),
        "ln1_w": np.ones(D, np.float32),
        "ln1_b": np.zeros(D, np.float32),
        "w_qkv": (rng.standard_normal((D, 3 * D), dtype=np.float32) / 32.0),
        "w_proj": (rng.standard_normal((D, D), dtype=np.float32) / 32.0),
        "ln2_w": np.ones(D, np.float32),
        "ln2_b": np.zeros(D, np.float32),
        "w_fc1": (rng.standard_normal((D, FF), dtype=np.float32) / 32.0),
        "w_fc2": (rng.standard_normal((FF, D), dtype=np.float32) / 64.0),
    }
    out = kernel(**ins)
    print("kernel out", out.shape, out.dtype, float(np.abs(out).mean()))


# revision 60
# speedup vs baseline: 1.3276x; 1.0175x over previous
"""Trainium2 Bass kernel for a dense transformer block (B=2, T=2048, D=1024, H=16).

Sharding (8 NeuronCores, one chip):
  - Token-split everywhere except attention: core i owns 512 tokens (rows
    512i:512i+512 of the flattened [4096, 1024] activation).
  - Head-split attention: core i owns heads {2i, 2i+1}.
  - Collectives (all AllToAll; no AllGather since AG is charged on its 8x
    output):
      #1  Q,K   (each core computes QKV for its own tokens, all heads, then
                 redistributes per-head)    [8, 256, 512] bf16
      #2  V     (token-major per k-tile)    [8, 512, 128] bf16
      #3  attention outputs back to token owners  [8, 128, 512] bf16
    #2 overlaps score compute; #1 overlaps V compute + residual transposes.

Layout is feature-major ("transposed", [feature, token]) on-chip; transposes
run on the PE (idle during the front-end) with DVE/ACT copies out of PSUM.
LN1 is folded into wqkv (w) and per-partition biases (b); LN2 is applied to
the residual stream BEFORE fc1 (w folded into wfc1, b as gelu bias), so the
MLP needs no per-h-tile unfold work on DVE. DMAs are few and large, spread
across the SP/ACT/Pool queues by dependency class (each queue sustains only
~1 transfer per ~3.6us in the cost model, and a DMA waiting on a collective
blocks everything behind it on the same queue).
"""

import numpy as np
import ml_dtypes

import concourse.bass as bass
import concourse.mybir as mybir
import concourse.tile as tile
from concourse import bacc
from concourse import bass_utils

F32 = mybir.dt.float32
MM_DT = mybir.dt.bfloat16
MM_NP = ml_dtypes.bfloat16

P = 128            # partitions
TB = 512           # tokens per core
D = 1024           # model dim
CT = D // P        # 8 feature tiles
NC = 8             # cores
TOKS = 2 * 2048    # global tokens
FF = 4096          # mlp hidden
DH = 64            # head dim
VW = 2 * (DH + 1)  # per-k-tile V columns: 2 heads x (64 dims + 1 ones col)
NKT = 32           # global k-tiles of 128 tokens

AF = mybir.ActivationFunctionType
OP = mybir.AluOpType
RG = [list(range(NC))]

ET_BUFS = 52       # et lookahead window (scores run ahead of AV by this much)

_cache: dict = {}


def _build():
    nc = bacc.Bacc(
        "TRN2",
        target_bir_lowering=False,
        debug=False,
        enable_asserts=False,
        num_devices=NC,
    )

    # ---- kernel I/O ----
    x_own = nc.dram_tensor("x_own", [TB, D], MM_DT, kind="ExternalInput").ap()
    wqkv = nc.dram_tensor("wqkv", [D, 3 * D], MM_DT, kind="ExternalInput").ap()
    wproj = nc.dram_tensor("wproj", [D, D], MM_DT, kind="ExternalInput").ap()
    wfc1 = nc.dram_tensor("wfc1", [D, FF], MM_DT, kind="ExternalInput").ap()
    wfc2 = nc.dram_tensor("wfc2", [FF, D], MM_DT, kind="ExternalInput").ap()
    fc1B = nc.dram_tensor("fc1B", [P, FF // P], F32, kind="ExternalInput").ap()
    bqkv = nc.dram_tensor("bqkv", [1, 3 * D], MM_DT, kind="ExternalInput").ap()
    bqkvc = nc.dram_tensor("bqkvc", [P, 2 * CT], F32, kind="ExternalInput").ap()
    cmask2 = nc.dram_tensor("cmask2", [P, 2 * P], F32, kind="ExternalInput").ap()
    id128m = nc.dram_tensor("id128m", [P, P], MM_DT, kind="ExternalInput").ap()
    out_t = nc.dram_tensor("out_t", [D, TB], F32, kind="ExternalOutput").ap()

    with tile.TileContext(nc) as tc:
        with (
            tc.tile_pool(name="persist", bufs=1) as pers,
            tc.tile_pool(name="dram", bufs=1, space="DRAM") as dram,
        ):
            # collective buffers
            qk_in = dram.tile([NC, 2 * P, TB], MM_DT, name="qk_in", tag="qk_in")
            qk_out = dram.tile([NC, 2 * P, TB], MM_DT, name="qk_out", tag="qk_out")
            v_in = dram.tile([NC, TB, P], MM_DT, name="v_in", tag="v_in")
            v_out = dram.tile([NC, TB, P], MM_DT, name="v_out", tag="v_out")
            y_in = dram.tile([NC, P, TB], MM_DT, name="y_in", tag="y_in")
            y_out = dram.tile([NC, P, TB], MM_DT, name="y_out", tag="y_out")

            # small constants
            ones_col_m = pers.tile([P, 1], MM_DT, name="ones_col_m", tag="ones_col_m")
            nc.gpsimd.memset(ones_col_m[:], 1.0)
            ones_row_m = pers.tile([1, DH], MM_DT, name="ones_row_m", tag="ones_row_m")
            nc.gpsimd.memset(ones_row_m[:], 1.0)
            ones_row_f = pers.tile([1, P], F32, name="ones_row_f", tag="ones_row_f")
            nc.gpsimd.memset(ones_row_f[:], 1.0)
            zb = pers.tile([P, 1], F32, name="zb", tag="zb")
            nc.gpsimd.memset(zb[:], 0.0)
            eps1 = pers.tile([1, 1], F32, name="eps1", tag="eps1")
            nc.gpsimd.memset(eps1[:], 1e-5)
            epsP = pers.tile([P, 1], F32, name="epsP", tag="epsP")
            nc.gpsimd.memset(epsP[:], 1e-5)
            cmask_sb = pers.tile([P, 2 * P], F32, name="cmask_sb", tag="cmask_sb")
            nc.gpsimd.dma_start(cmask_sb[:], cmask2[:])
            idm_sb = pers.tile([P, P], MM_DT, name="idm_sb", tag="idm_sb")
            nc.gpsimd.dma_start(idm_sb[:], id128m[:])
            fc1B_sb = pers.tile([P, FF // P], F32, name="fc1B_sb", tag="fc1B_sb")
            bqkvc_sb = pers.tile([P, 2 * CT], F32, name="bqkvc_sb", tag="bqkvc_sb")

            # persistent activations
            xT = pers.tile([P, CT * TB], F32, name="xT", tag="xT")  # raw x, feat-major
            x2T = [pers.tile([P, TB], F32, name=f"x2T{c}", tag=f"x2T{c}")
                   for c in range(CT)]
            qT = pers.tile([P, TOKS], MM_DT, name="qT", tag="qT")
            kT = pers.tile([P, TOKS], MM_DT, name="kT", tag="kT")
            V_sb = pers.tile([P, NKT * VW], MM_DT, name="V_sb", tag="V_sb")
            # ones columns (data columns DMA'd in later)
            nc.gpsimd.memset(
                V_sb[:].rearrange("p (g w) -> p g w", w=DH + 1)[:, :, DH:DH + 1], 1.0)
            x2m = [pers.tile([P, TB], MM_DT, name=f"x2m{c}", tag=f"x2m{c}")
                   for c in range(CT)]
            wproj_sb = pers.tile([P, CT * D], MM_DT, name="wproj_sb", tag="wproj_sb")

            # ============ Phase A: load, LN1, QKV, A2A #1/#2 ============
            with (
                tc.tile_pool(name="ps_a", bufs=6, space="PSUM") as ps_a,
                tc.tile_pool(name="ps_t", bufs=2, space="PSUM") as ps_t,
                tc.tile_pool(name="work_a", bufs=2) as work_a,
                tc.tile_pool(name="xrow_a", bufs=1) as xrow_a,
                tc.tile_pool(name="wqkv_a", bufs=1) as wqkv_a,
                tc.tile_pool(name="xn_a", bufs=1) as xn_a,
                tc.tile_pool(name="qkl_a", bufs=1) as qkl_a,
                tc.tile_pool(name="vl_a", bufs=1) as vl_a,
            ):
                # single big DMAs: per-queue throughput is the scarce resource
                bqkv_sb = xrow_a.tile([1, 3 * D], MM_DT, name="bqkv_sb",
                                      tag="bqkv_sb")
                ones_row_p = xrow_a.tile([1, P], MM_DT, name="ones_row_p",
                                         tag="ones_row_p")
                nc.gpsimd.memset(ones_row_p[:], 1.0)
                xrow = xrow_a.tile([P, 4 * D], MM_DT, name="xrow", tag="xrow")
                nc.sync.dma_start(
                    xrow[:].rearrange("p (r c) -> p r c", r=4),
                    x_own[:].rearrange("(r p) c -> p r c", p=P))
                wqkv_sb = wqkv_a.tile([P, CT * 3 * D], MM_DT, name="wqkv_sb",
                                      tag="wqkv_sb")

                def wq(c):  # [128, 3*D] block of k-tile c
                    return wqkv_sb[:, c * 3 * D:(c + 1) * 3 * D]

                for half in range(2):
                    nc.sync.dma_start(
                        wqkv_sb[:, half * 4 * 3 * D:(half + 1) * 4 * 3 * D]
                        .rearrange("p (c j) -> p c j", c=4),
                        wqkv[half * 4 * P:(half + 1) * 4 * P, :]
                        .rearrange("(c p) j -> p c j", p=P))
                nc.scalar.dma_start(bqkv_sb[:], bqkv[:])
                nc.scalar.dma_start(bqkvc_sb[:], bqkvc[:])
                nc.gpsimd.dma_start(fc1B_sb[:], fc1B[:])

                # LN1 per row-block (natural layout, bf16; ln1_w folded into
                # wqkv, ln1_b via bias terms). DVE-only chain except the Sqrt.
                xnn_l = [xn_a.tile([P, D], MM_DT, name=f"xnn{r}", tag=f"xnn{r}")
                         for r in range(TB // P)]
                for r in range(TB // P):
                    xr = xrow[:, r * D:(r + 1) * D]
                    s1 = work_a.tile([P, 1], F32, name=f"s1_{r}", tag="lns1")
                    nc.vector.tensor_reduce(s1[:], xr,
                                            axis=mybir.AxisListType.X, op=OP.add)
                    sqf = work_a.tile([P, D], MM_DT, name=f"sq_{r}", tag="lnsq")
                    s2 = work_a.tile([P, 1], F32, name=f"s2_{r}", tag="lns2")
                    nc.vector.scalar_tensor_tensor(
                        out=sqf[:], in0=xr, scalar=1.0, in1=xr,
                        op0=OP.mult, op1=OP.mult, accum_out=s2[:])
                    mu = work_a.tile([P, 1], F32, name=f"mu_{r}", tag="lnmu")
                    nc.vector.tensor_scalar_mul(mu[:], s1[:], 1.0 / D)
                    mu2 = work_a.tile([P, 1], F32, name=f"mu2_{r}", tag="lnmu2")
                    nc.vector.tensor_mul(mu2[:], mu[:], mu[:])
                    var = work_a.tile([P, 1], F32, name=f"var_{r}", tag="lnvar")
                    nc.vector.scalar_tensor_tensor(
                        out=var[:], in0=s2[:], scalar=1.0 / D, in1=mu2[:],
                        op0=OP.mult, op1=OP.subtract)
                    sd = work_a.tile([P, 1], F32, name=f"sd_{r}", tag="lnsd")
                    nc.scalar.activation(sd[:], var[:], AF.Sqrt, bias=epsP[:])
                    rstd = work_a.tile([P, 1], F32, name=f"rstd_{r}", tag="lnrstd")
                    nc.vector.reciprocal(rstd[:], sd[:])
                    nc.vector.tensor_scalar(xnn_l[r][:], xr, mu[:], rstd[:],
                                            OP.subtract, OP.mult)

                # transposes on the (idle) PE; copies split across DVE/ACT
                xnT = xn_a.tile([P, CT * TB], MM_DT, name="xnT", tag="xnT")
                for r in range(TB // P):
                    for c in range(CT):
                        pt = ps_t.tile([P, P], MM_DT, name=f"ptn_{r}_{c}",
                                       tag="ptn")
                        nc.tensor.transpose(
                            pt[:], xnn_l[r][:, c * P:(c + 1) * P], idm_sb[:])
                        nc.vector.tensor_copy(
                            xnT[:, c * TB + r * P:c * TB + (r + 1) * P], pt[:])

                # ---- Q,K matmuls (feature-major, my 512 tokens) ----
                # c-major in groups so the PE streams while later weight
                # chunks are still in flight
                qkl = qkl_a.tile([P, NC * 2 * TB], MM_DT, name="qkl", tag="qkl")
                for g in range(2):  # 0=q, 1=k
                    ps_l = [ps_a.tile([P, TB], F32, name=f"qk{d}_{g}", tag="psqk")
                            for d in range(NC)]
                    for c in range(CT):
                        for d in range(NC):
                            nc.tensor.matmul(
                                ps_l[d][:], wq(c)[:, g * D + d * P:
                                                  g * D + (d + 1) * P],
                                xnT[:, c * TB:(c + 1) * TB],
                                start=(c == 0), stop=(c == CT - 1))
                    for d in range(NC):
                        # ln1_b contribution (b @ W) as per-partition bias
                        nc.scalar.activation(
                            qkl[:, (2 * d + g) * TB:(2 * d + g + 1) * TB],
                            ps_l[d][:], AF.Identity,
                            bias=bqkvc_sb[:, g * CT + d:g * CT + d + 1])
                    if g == 0:
                        nc.scalar.dma_start(
                            qk_in[:, 0:P, :].rearrange("d p t -> p d t"),
                            qkl[:].rearrange("p (d gg t) -> p d gg t", gg=2, t=TB)
                            [:, :, 0, :])
                    else:
                        nc.sync.dma_start(
                            qk_in[:, P:2 * P, :].rearrange("d p t -> p d t"),
                            qkl[:].rearrange("p (d gg t) -> p d gg t", gg=2, t=TB)
                            [:, :, 1, :])
                nc.gpsimd.collective_compute(
                    "AllToAll", OP.bypass, replica_groups=RG,
                    ins=[qk_in[:]], outs=[qk_out[:]])

                # ---- V matmuls (token-major) + A2A #2 ----
                vloc = vl_a.tile([P, 4 * D], MM_DT, name="vloc", tag="vloc")
                for tt in range(TB // P):
                    for nh in range(2):
                        ps = ps_a.tile([P, TB], F32, name=f"v{tt}_{nh}", tag="psqk")
                        nc.tensor.matmul(
                            ps[:], ones_row_p[:],
                            bqkv_sb[0:1, 2 * D + nh * TB:2 * D + (nh + 1) * TB],
                            start=True, stop=False)
                        for c in range(CT):
                            nc.tensor.matmul(
                                ps[:],
                                xnT[:, c * TB + tt * P:c * TB + (tt + 1) * P],
                                wq(c)[:, 2 * D + nh * TB:2 * D + (nh + 1) * TB],
                                start=False, stop=(c == CT - 1))
                        # write dest-major: vloc[p, (d*4 + tt)*P + c]
                        nc.scalar.activation(
                            vloc[:].rearrange("p (q tt c) -> p q tt c",
                                              tt=4, c=P)
                            [:, nh * 4:(nh + 1) * 4, tt, :],
                            ps[:], AF.Copy)
                nc.sync.dma_start(
                    v_in[:].rearrange("d (tt p) c -> p (d tt) c", p=P),
                    vloc[:].rearrange("p (dt c) -> p dt c", c=P))
                nc.gpsimd.collective_compute(
                    "AllToAll", OP.bypass, replica_groups=RG,
                    ins=[v_in[:]], outs=[v_out[:]])

                # ---- residual-path transposes of raw x (fills A2A wait) ----
                for r in range(TB // P):
                    for c in range(CT):
                        pt = ps_t.tile([P, P], MM_DT, name=f"ptr_{r}_{c}",
                                       tag="ptn")
                        nc.tensor.transpose(
                            pt[:], xrow[:, r * D + c * P:r * D + (c + 1) * P],
                            idm_sb[:])
                        nc.scalar.activation(
                            xT[:, c * TB + r * P:c * TB + (r + 1) * P],
                            pt[:], AF.Copy)
                for half in range(2):
                    nc.scalar.dma_start(
                        wproj_sb[:, half * 4 * D:(half + 1) * 4 * D]
                        .rearrange("p (c j) -> p c j", c=4),
                        wproj[half * 4 * P:(half + 1) * 4 * P, :]
                        .rearrange("(c p) j -> p c j", p=P))

            # ============ Phase B: attention ============
            with (
                tc.tile_pool(name="ps_s", bufs=3, space="PSUM") as ps_s_pool,
                tc.tile_pool(name="ps_y", bufs=1, space="PSUM") as ps_y_pool,
                tc.tile_pool(name="epool", bufs=ET_BUFS) as epool,
                tc.tile_pool(name="work_b", bufs=4) as work_b,
                tc.tile_pool(name="rb_b", bufs=2) as rb_b,
            ):
                # receive q/k (feature-major, all tokens for my 2 heads);
                # batch-0 sources first so b=0 scores start immediately
                for half in range(2):
                    sl = slice(half * 4, (half + 1) * 4)
                    nc.sync.dma_start(
                        kT[:, half * 4 * TB:(half + 1) * 4 * TB]
                        .rearrange("p (s t) -> p s t", s=4),
                        qk_out[sl, P:2 * P, :].rearrange("s p t -> p s t"))
                    nc.scalar.dma_start(
                        qT[:, half * 4 * TB:(half + 1) * 4 * TB]
                        .rearrange("p (s t) -> p s t", s=4),
                        qk_out[sl, 0:P, :].rearrange("s p t -> p s t"))
                # receive V into packed [tok, (head, dh|one)] slots
                for h in range(2):
                    nc.sync.dma_start(
                        V_sb[:].rearrange("p (gk hh w) -> p gk hh w",
                                          w=DH + 1, hh=2)[:, :, h, 0:DH],
                        v_out[:].rearrange("s (tt p) (hh c) -> p (s tt) hh c",
                                           p=P, c=DH)[:, :, h, :])

                # score/exp jobs and AV jobs, interleaved with a lookahead
                # window so the PE never sits behind a V-gated AV while
                # score work remains.
                sjobs = []   # (b, j, kt, pss, et)
                for b in range(2):
                    for j in range(4):
                        for kt in range(4 * j + 4):
                            sjobs.append([b, j, kt, None, None])

                def emit_score(job):
                    b, j, kt, _, _ = job
                    jg = b * 4 + j
                    qoff = jg * TB
                    gk = b * 16 + kt
                    n0 = 0 if kt < 4 * j else (kt - 4 * j) * P
                    pss = ps_s_pool.tile([P, 2 * TB], F32,
                                         name=f"pss{b}_{j}_{kt}", tag="pss")
                    for h in range(2):
                        nc.tensor.matmul(
                            pss[:, h * TB + n0:(h + 1) * TB],
                            kT[h * DH:(h + 1) * DH, gk * P:(gk + 1) * P],
                            qT[h * DH:(h + 1) * DH, qoff + n0:qoff + TB],
                            start=True, stop=True,
                            tile_position=(h * DH, 0))
                    if kt >= 4 * j:
                        nc.vector.tensor_add(
                            pss[:].rearrange("p (h t) -> p h t", h=2)
                            [:, :, n0:n0 + P],
                            pss[:].rearrange("p (h t) -> p h t", h=2)
                            [:, :, n0:n0 + P],
                            cmask_sb[:].rearrange("p (h t) -> p h t", h=2))
                    et = epool.tile([P, 2 * TB], MM_DT,
                                    name=f"et{b}_{j}_{kt}", tag="et")
                    if n0 == 0:
                        nc.scalar.activation(et[:], pss[:], AF.Exp, bias=zb[:])
                    else:
                        nc.scalar.activation(
                            et[:].rearrange("p (h t) -> p h t", h=2)[:, :, n0:TB],
                            pss[:].rearrange("p (h t) -> p h t", h=2)[:, :, n0:TB],
                            AF.Exp, bias=zb[:])
                    job[3] = pss
                    job[4] = et

                si = 0
                for si in range(min(ET_BUFS - 2, len(sjobs))):
                    emit_score(sjobs[si])
                si += 1

                for b in range(2):
                    for j in range(4):
                        jg = b * 4 + j
                        nkt = 4 * j + 4
                        ps_y = [ps_y_pool.tile([DH + 1, TB], F32,
                                               name=f"psy{b}_{j}_{h}", tag=f"psy{h}")
                                for h in range(2)]
                        base = sum(4 * (g % 4) + 4 for g in range(4 * b + j))
                        for kt in range(nkt):
                            job = sjobs[base + kt]
                            _, _, _, pss, et = job
                            n0 = 0 if kt < 4 * j else (kt - 4 * j) * P
                            gk = b * 16 + kt
                            for h in range(2):
                                vsl = V_sb[:, gk * VW + h * (DH + 1):
                                           gk * VW + h * (DH + 1) + DH + 1]
                                nc.tensor.matmul(
                                    ps_y[h][:, n0:TB], vsl,
                                    et[:, h * TB + n0:(h + 1) * TB],
                                    start=(kt == 0), stop=(kt == nkt - 1))
                            if si < len(sjobs):
                                emit_score(sjobs[si])
                                si += 1
                        # normalize: y / colsum -> bf16 -> a2a staging
                        yn = work_b.tile([P, TB], MM_DT, name=f"yn{b}_{j}",
                                         tag="yn")
                        for h in range(2):
                            rf = work_b.tile([1, TB], F32, name=f"rf{b}_{j}_{h}",
                                             tag="rf")
                            nc.vector.reciprocal(rf[:], ps_y[h][DH:DH + 1, :])
                            # denominator broadcast on the idle GPSIMD
                            rb = rb_b.tile([DH, TB], F32, name=f"rb{b}_{j}_{h}",
                                           tag="rb")
                            nc.gpsimd.partition_broadcast(rb[:], rf[:],
                                                          channels=DH)
                            nc.vector.tensor_mul(
                                yn[h * DH:(h + 1) * DH, :],
                                ps_y[h][0:DH, :], rb[:])
                        nc.sync.dma_start(y_in[jg], yn[:])

                nc.gpsimd.collective_compute(
                    "AllToAll", OP.bypass, replica_groups=RG,
                    ins=[y_in[:]], outs=[y_out[:]])

            # ============ Phase C: proj + residual + LN2 ============
            with (
                tc.tile_pool(name="ps_c", bufs=2, space="PSUM") as ps_c,
                tc.tile_pool(name="ps_ln", bufs=2, space="PSUM") as ps_ln,
                tc.tile_pool(name="ps_bc", bufs=2, space="PSUM") as ps_bc,
                tc.tile_pool(name="work_c", bufs=2) as work_c,
                tc.tile_pool(name="yall_c", bufs=1) as yall_c,
            ):
                yall = yall_c.tile([P, NC * TB], MM_DT, name="yall", tag="yall")
                nc.scalar.dma_start(
                    yall[:, 0:4 * TB].rearrange("p (s t) -> p s t", s=4),
                    y_out[0:4].rearrange("s p t -> p s t"))
                nc.sync.dma_start(
                    yall[:, 4 * TB:].rearrange("p (s t) -> p s t", s=4),
                    y_out[4:8].rearrange("s p t -> p s t"))
                s1 = ps_ln.tile([1, TB], F32, name="ln2s1", tag="lnsum")
                s2 = ps_ln.tile([1, TB], F32, name="ln2s2", tag="lnsum")
                for c in range(CT):
                    ps = ps_c.tile([P, TB], F32, name=f"proj{c}", tag="projps")
                    for s in range(NC):
                        nc.tensor.matmul(
                            ps[:],
                            wproj_sb[:, s * D + c * P:s * D + (c + 1) * P],
                            yall[:, s * TB:(s + 1) * TB],
                            start=(s == 0), stop=(s == NC - 1))
                    nc.vector.tensor_add(x2T[c][:], ps[:],
                                         xT[:, c * TB:(c + 1) * TB])
                    # LN2 stats via ones-matmuls on a bf16 scratch copy
                    x2s = work_c.tile([P, TB], MM_DT, name=f"x2s{c}", tag="x2s")
                    nc.scalar.activation(x2s[:], x2T[c][:], AF.Copy)
                    nc.tensor.matmul(s1[:], ones_col_m[:], x2s[:],
                                     start=(c == 0), stop=(c == CT - 1))
                    sq = work_c.tile([P, TB], MM_DT, name=f"sq{c}", tag="ln2sq")
                    nc.vector.tensor_mul(sq[:], x2s[:], x2s[:])
                    nc.tensor.matmul(s2[:], ones_col_m[:], sq[:],
                                     start=(c == 0), stop=(c == CT - 1))
                mu = work_c.tile([1, TB], F32, name="ln2mu", tag="ln2mu")
                nc.vector.tensor_scalar_mul(mu[:], s1[:], 1.0 / D)
                mu2 = work_c.tile([1, TB], F32, name="ln2mu2", tag="ln2mu2")
                nc.vector.tensor_mul(mu2[:], mu[:], mu[:])
                vr = work_c.tile([1, TB], F32, name="ln2vr", tag="ln2vr")
                nc.vector.scalar_tensor_tensor(
                    out=vr[:], in0=s2[:], scalar=1.0 / D, in1=mu2[:],
                    op0=OP.mult, op1=OP.subtract)
                sdr = work_c.tile([1, TB], F32, name="ln2sd", tag="ln2sd")
                nc.scalar.activation(sdr[:], vr[:], AF.Sqrt, bias=eps1[:])
                rs = work_c.tile([1, TB], F32, name="ln2rs", tag="ln2rs")
                nc.vector.reciprocal(rs[:], sdr[:])
                mrow = work_c.tile([1, TB], F32, name="ln2mr", tag="ln2mr")
                nc.vector.tensor_mul(mrow[:], mu[:], rs[:])
                # broadcast rstd / mu*rstd on GPSIMD (SBUF-resident), then
                # split the normalize across DVE and GPSIMD
                rs_f = work_c.tile([P, TB], F32, name="rs_f", tag="rs_f")
                nc.gpsimd.partition_broadcast(rs_f[:], rs[:], channels=P)
                m_f = work_c.tile([P, TB], F32, name="m_f", tag="m_f")
                nc.gpsimd.partition_broadcast(m_f[:], mrow[:], channels=P)
                for c in range(CT):
                    if c < 5:
                        t1 = work_c.tile([P, TB], F32, name=f"xn2a{c}", tag="xn2a")
                        nc.vector.tensor_mul(t1[:], x2T[c][:], rs_f[:])
                        nc.vector.tensor_sub(x2m[c][:], t1[:], m_f[:])
                    else:
                        t1 = work_c.tile([P, TB], F32, name=f"xn2a{c}", tag="xn2b")
                        nc.gpsimd.tensor_mul(t1[:], x2T[c][:], rs_f[:])
                        nc.gpsimd.tensor_sub(x2m[c][:], t1[:], m_f[:])

            # ============ Phase D/E: fc1 + gelu, fc2 + residual ============
            HB = 4  # h-tiles per fc1 weight block
            with (
                tc.tile_pool(name="w1pool", bufs=1) as w1pool,
                tc.tile_pool(name="w2pool", bufs=1) as w2pool,
                tc.tile_pool(name="g1pool", bufs=1) as g1pool,
                tc.tile_pool(name="x3pool", bufs=1) as x3pool,
            ):
                g1 = [g1pool.tile([P, TB], MM_DT, name=f"g1_{h}", tag=f"g1_{h}")
                      for h in range(FF // P)]
                w2_all = w2pool.tile([P, (FF // P) * D], MM_DT, name="w2_all",
                                     tag="w2_all")
                w1t_l = [w1pool.tile([P, CT * HB * P], MM_DT, name=f"w1t{blk}",
                                     tag=f"w1t{blk % 3}")
                         for blk in range(FF // (HB * P))]
                # weight streams on the gpsimd queue: first two fc1 blocks,
                # then wfc2 chunks interleaved with the remaining fc1 blocks
                def load_w1(blk):
                    nc.scalar.dma_start(
                        w1t_l[blk][:].rearrange("p (c h) -> p c h", c=CT),
                        wfc1[:, blk * HB * P:(blk + 1) * HB * P]
                        .rearrange("(c p) h -> p c h", p=P))

                def load_w2(q):
                    nc.scalar.dma_start(
                        w2_all[:, q * 4 * D:(q + 1) * 4 * D]
                        .rearrange("p (ht j) -> p ht j", ht=4),
                        wfc2[q * 4 * P:(q + 1) * 4 * P, :]
                        .rearrange("(ht p) j -> p ht j", p=P))
                load_w1(0)
                load_w1(1)
                for blk in range(2, FF // (HB * P)):
                    for q in (2 * (blk - 2), 2 * (blk - 2) + 1):
                        if q < 8:
                            load_w2(q)
                    load_w1(blk)
                with tc.tile_pool(name="ps_fc1", bufs=2, space="PSUM") as ps_fc1:
                    for blk in range(FF // (HB * P)):
                        w1t = w1t_l[blk]
                        for hh in range(HB):
                            ht = blk * HB + hh
                            ps = ps_fc1.tile([P, TB], F32, name=f"fc1_{ht}",
                                             tag="fc1ps")
                            for c in range(CT):
                                nc.tensor.matmul(
                                    ps[:],
                                    w1t[:, c * HB * P + hh * P:
                                        c * HB * P + (hh + 1) * P],
                                    x2m[c][:],
                                    start=(c == 0), stop=(c == CT - 1))
                            nc.scalar.activation(g1[ht][:], ps[:], AF.Gelu,
                                                 bias=fc1B_sb[:, ht:ht + 1])

                with tc.tile_pool(name="ps_fc2", bufs=2, space="PSUM") as ps_fc2:
                    for c in range(CT):
                        ps = ps_fc2.tile([P, TB], F32, name=f"fc2_{c}", tag="fc2ps")
                        for ht in range(FF // P):
                            nc.tensor.matmul(
                                ps[:], w2_all[:, ht * D + c * P:ht * D + (c + 1) * P],
                                g1[ht][:],
                                start=(ht == 0), stop=(ht == FF // P - 1))
                        x3 = x3pool.tile([P, TB], F32, name=f"x3_{c}", tag="x3")
                        nc.vector.tensor_add(x3[:], ps[:], x2T[c][:])
                        nc.sync.dma_start(out_t[c * P:(c + 1) * P, :], x3[:])

    nc.compile()
    return nc


def _prep_inputs(x, ln1_w, ln1_b, w_qkv, w_proj, ln2_w, ln2_b, w_fc1, w_fc2):
    xf = np.ascontiguousarray(np.asarray(x, np.float32).reshape(TOKS, D))
    # fold ln1_w into wqkv rows; ln1_b contributes the rank-1 bqkv term
    ln1w_f = np.asarray(ln1_w, np.float32)
    ln1b_f = np.asarray(ln1_b, np.float32)
    wq = np.asarray(w_qkv[:, :D], np.float32) * 0.125  # fold 1/sqrt(dh)
    wk = np.asarray(w_qkv[:, D:2 * D], np.float32)
    wv = np.asarray(w_qkv[:, 2 * D:], np.float32)
    wqkv_f = np.concatenate([wq, wk, wv], axis=1)
    bqkv_full = ln1b_f @ wqkv_f
    bqkv_np = bqkv_full.reshape(1, 3 * D).astype(MM_NP)
    # per-partition column layout of the q,k parts for the ACT-copy bias
    bqkvc_np = np.ascontiguousarray(
        bqkv_full[:2 * D].reshape(2 * CT, P).T.astype(np.float32))
    wqkv_np = (ln1w_f[:, None] * wqkv_f).astype(MM_NP)
    wproj_np = np.asarray(w_proj, np.float32).astype(MM_NP)
    wfc2_np = np.asarray(w_fc2, np.float32).astype(MM_NP)
    # LN2 folded into fc1: scale wfc1 rows by ln2_w; B = ln2_b @ w_fc1;
    # A = -colsum of the bf16-rounded weights (matches the device matmul)
    w1p = (np.asarray(ln2_w, np.float32)[:, None] *
           np.asarray(w_fc1, np.float32))
    wfc1_np = w1p.astype(MM_NP)
    Bv = np.asarray(ln2_b, np.float32) @ np.asarray(w_fc1, np.float32)
    fc1B_np = np.ascontiguousarray(Bv.reshape(FF // P, P).T)
    pp, jj = np.meshgrid(np.arange(P), np.arange(P), indexing="ij")
    cm = np.where(pp <= jj, 0.0, -1e30).astype(np.float32)
    cmask2_np = np.ascontiguousarray(np.tile(cm, (1, 2)))
    common = {
        "wqkv": wqkv_np, "wproj": wproj_np, "wfc1": wfc1_np, "wfc2": wfc2_np,
        "fc1B": fc1B_np, "bqkv": bqkv_np, "bqkvc": bqkvc_np,
        "cmask2": cmask2_np,
        "id128m": np.eye(P, dtype=np.float32).astype(MM_NP),
    }
    in_maps = []
    for i in range(NC):
        m = dict(common)
        m["x_own"] = np.ascontiguousarray(xf[TB * i:TB * (i + 1)]).astype(MM_NP)
        in_maps.append(m)
    return in_maps


def _get_runner():
    """Build (once) a cached, non-donating PJRT executable for the kernel."""
    if "runner" in _cache:
        return _cache["runner"]
    import jax
    from jax.sharding import Mesh, PartitionSpec, NamedSharding
    from jax.experimental.shard_map import shard_map
    from concourse import bass2jax

    nc = _cache.get("nc")
    if nc is None:
        nc = _cache["nc"] = _build()
    bass2jax.install_neuronx_cc_hook()
    partition_name = nc.partition_id_tensor.name if nc.partition_id_tensor else None
    in_names, out_names, out_avals, zero_outs = [], [], [], []
    for alloc in nc.m.functions[0].allocations:
        if not isinstance(alloc, mybir.MemoryLocationSet):
            continue
        name = alloc.memorylocations[0].name
        if alloc.kind == "ExternalInput":
            if name != partition_name:
                in_names.append(name)
        elif alloc.kind == "ExternalOutput":
            out_names.append(name)
            shape = tuple(alloc.tensor_shape)
            dtype = mybir.dt.np(alloc.dtype)
            out_avals.append(jax.core.ShapedArray(shape, dtype))
            zero_outs.append(np.zeros(shape, dtype))
    n_params = len(in_names)
    all_in_names = in_names + out_names + ([partition_name] if partition_name else [])

    def _body(*args):
        operands = list(args)
        if partition_name is not None:
            operands.append(bass2jax.partition_id_tensor())
        outs = bass2jax._bass_exec_p.bind(
            *operands, out_avals=tuple(out_avals), in_names=tuple(all_in_names),
            out_names=tuple(out_names), lowering_input_output_aliases=(),
            sim_require_finite=True, sim_require_nnan=True, nc=nc)
        return tuple(outs)

    devices = jax.devices()[:NC]
    mesh = Mesh(np.asarray(devices), ("core",))
    nin = n_params + len(out_names)
    sharded = jax.jit(shard_map(
        _body, mesh=mesh, in_specs=(PartitionSpec("core"),) * nin,
        out_specs=(PartitionSpec("core"),) * len(out_names), check_rep=False))
    sh = NamedSharding(mesh, PartitionSpec("core"))
    dev_zeros = [
        jax.device_put(np.zeros((NC * z.shape[0], *z.shape[1:]), z.dtype), sh)
        for z in zero_outs
    ]
    runner = (sharded, in_names, out_names, out_avals, sh, dev_zeros)
    _cache["runner"] = runner
    return runner


def kernel(**inputs):
    import jax
    sharded, in_names, out_names, out_avals, sh, dev_zeros = _get_runner()
    in_maps = _prep_inputs(**inputs)
    concat_in = [np.concatenate([in_maps[c][nm] for c in range(NC)], axis=0)
                 for nm in in_names]
    dev_in = [jax.device_put(a, sh) for a in concat_in]
    out_arrs = sharded(*dev_in, *dev_zeros)
    got = {nm: np.asarray(out_arrs[i]).reshape(NC, *out_avals[i].shape)
           for i, nm in enumerate(out_names)}
    out = np.empty((TOKS, D), np.float32)
    for i in range(NC):
        out[TB * i:TB * (i + 1)] = got["out_t"][i].T
    return out.reshape(2, 2048, D)


if __name__ == "__main__":
    rng = np.random.default_rng(0)
    ins = {
        "x": rng.standard_normal((2, 2048, D), dtype=np.float32),
        "ln1_w": np.ones(D, np.float32),
        "ln1_b": np.zeros(D, np.float32),
        "w_qkv": (rng.standard_normal((D, 3 * D), dtype=np.float32) / 32.0),
        "w_proj": (rng.standard_normal((D, D), dtype=np.float32) / 32.0),
        "ln2_w": np.ones(D, np.float32),
        "ln2_b": np.zeros(D, np.float32),
        "w_fc1": (rng.standard_normal((D, FF), dtype=np.float32) / 32.0),
        "w_fc2": (rng.standard_normal((FF, D), dtype=np.float32) / 64.0),
    }
    out = kernel(**ins)
    print("kernel out", out.shape, out.dtype, float(np.abs(out).mean()))
